# revision 34
# baseline (speedup 1.0000x reference)
"""MultiHeadSelfAttention (qk-LayerNorm variant) on 8 TRN2 NeuronCores. v3.

v3 changes (vs v2, 728us):
  * P1 LN chain moved off the critical path: rstd via ACT ln/exp (ACT is
    idle in P1), normalize via ACT Identity with per-partition scale/bias
    writing bf16 tok, and the Q^T/K^T transposes via DMA xbar (bf16)
    straight into residents -- P1 loses all PE transposes, PSUM transpose
    tiles, and DVE sink copies (~1.8us/tile PE stall in v2's trace).
  * kp/qnT now bf16 (halves SBUF + transpose traffic; S matmul numerics
    fine post-LN).
  * K-group weights prefetched during Q phase (wq pool bufs=2).
  * Softmax normalize overlapped into P2b: reciprocal batch A (pairs 0-3)
    after pair 3, blocks normalized in the pair gaps via po-tag reuse;
    batch B at tail. P2c phase gone; P3 entry stall (~15us) gone.

Problem (B=4, N=2048, C=1024, H=16, D=64, fp32):
    qkv = x @ W_qkv + b_qkv ; q,k,v = split(qkv)
    q = LN(q)*scale ; k = LN(k)          (LN over full C)
    attn = softmax(q_h @ k_h^T) per head ; o = attn @ v_h
    out = concat_heads(o) @ W_proj + b_proj

Sharding: core i handles batch i//2 and query-half i%2 (1024 query rows).
Each core computes K/V for the full sequence of its batch. No collectives.

v2 design notes (vs v1):
  * ACT (scalar engine) runs ONLY Exp/Log: softmax exp is the hard floor
    (1 elem/cycle/lane, ~285us/core), so LN stats moved to DVE bn_stats
    and rstd/reciprocal computed as exp(-0.5*ln(var+eps)) / exp(-ln(s))
    on ACT -- all in the natural_log_exp table set, zero table thrash.
  * K^T SBUF-resident (64KB/part); V staged to DRAM as bf16 aug tiles
    [64 v | ones | 0] (66 wide, even for bf16 ISA), reloaded per pair.
  * S^T matmuls fp32r with h0 (rows 0-63) / h1 (rows 64-127) issued
    back-to-back: PE row-group concurrency gives ~2x (measured 147ns/MM).
  * PV + projection in bf16 (safe: post-softmax averaging; numerator and
    denominator share pT rounding).
  * pair0-h0 attention fused into the V-production phase so exp starts
    while V tiles are still being produced.
  * Softmax normalization deferred: unnormalized O^T and row-sums are
    saved per (pair,h2); reciprocals batched on ACT, broadcast via K=1
    rank-1 matmuls, applied with one DVE mult per head before proj.
"""
import numpy as np
import ml_dtypes
from contextlib import ExitStack

import concourse.bass as bass


# NOTE: do NOT override BASS_ACT_ROOT_JSON_PATH to merge the ln/exp table
# sets -- the runtime resolves PSEUDO_LOAD_ACT_FUNC_SET against its own
# stock registry and a reordered json hangs the NEFF (mesh desync).
from concourse import bacc
import concourse.tile as tile
import concourse.mybir as mybir
from concourse.masks import make_identity

dt = mybir.dt
AF = mybir.ActivationFunctionType
OP = mybir.AluOpType
ts = bass.ts

B, N, C = 4, 2048, 1024
H, D = 16, 64
NQ = 1024            # query rows per core
SCALE = D ** -0.5
EPS = 1e-6
TT = N // 128        # 16 token tiles (full seq)
TQ = NQ // 128       # 8 token tiles (query half)
CT = C // 128        # 8 channel tiles (= head pairs)
F32R = dt.float32r
BF16 = dt.bfloat16


def build_nc(reps=1, with_bias=False, trivial_affine=True):
    nc = bacc.Bacc()
    xTt = nc.dram_tensor("xTt", [TT, 128, C], dt.float32, kind="ExternalInput")
    xTqt = nc.dram_tensor("xTqt", [TQ, 128, C], dt.float32, kind="ExternalInput")
    wqkv = nc.dram_tensor("wqkv", [C, 3 * C], dt.float32, kind="ExternalInput")
    wprojb = nc.dram_tensor("wprojb", [C, C], BF16, kind="ExternalInput")
    bqkv = nc.dram_tensor("bqkv", [3 * C], dt.float32, kind="ExternalInput")
    bprojb = nc.dram_tensor("bprojb", [C], BF16, kind="ExternalInput")
    ones128 = nc.dram_tensor("ones128", [128], dt.float32, kind="ExternalInput")
    kron48 = nc.dram_tensor("kron48", [16, 1024], BF16, kind="ExternalInput")
    gq = nc.dram_tensor("gq", [C], dt.float32, kind="ExternalInput")
    bq = nc.dram_tensor("bq", [C], dt.float32, kind="ExternalInput")
    gk = nc.dram_tensor("gk", [C], dt.float32, kind="ExternalInput")
    bk = nc.dram_tensor("bk", [C], dt.float32, kind="ExternalInput")
    out = nc.dram_tensor("out", [NQ, C], dt.float32, kind="ExternalOutput")

    with tile.TileContext(nc) as tc, ExitStack() as top:
        const = top.enter_context(tc.tile_pool(name="const", bufs=1))
        dram = top.enter_context(tc.tile_pool(name="dram", bufs=1, space="DRAM"))
        res = top.enter_context(tc.tile_pool(name="res", bufs=1))

        # ---- constants ----
        ident = const.tile([128, 128], dt.float32)
        make_identity(nc, ident[:])
        identb = const.tile([128, 128], BF16)
        nc.vector.tensor_copy(identb[:], ident[:])
        ones1 = const.tile([1, 128], F32R)
        nc.sync.dma_start(ones1[:], ones128.rearrange("(o n) -> o n", o=1)
                          .bitcast(F32R))
        ones1b = const.tile([1, 128], BF16)
        nc.vector.tensor_copy(ones1b[:], ones1[:].bitcast(dt.float32))
        ones16 = const.tile([128, 16], dt.float32)
        nc.vector.memset(ones16[:], 1.0)
        eps_t = const.tile([128, 1], dt.float32)
        nc.vector.memset(eps_t[:], EPS)
        epsq_t = const.tile([128, 1], dt.float32)
        nc.vector.memset(epsq_t[:], EPS / SCALE ** 2)
        gq_t = const.tile([128, CT], dt.float32)
        bq_t = const.tile([128, CT], dt.float32)
        gk_t = const.tile([128, CT], dt.float32)
        bk_t = const.tile([128, CT], dt.float32)
        for t_, d_ in ((gq_t, gq), (bq_t, bq), (gk_t, gk), (bk_t, bk)):
            nc.sync.dma_start(t_[:], d_.rearrange("(ct p) -> p ct", p=128))
        if with_bias:
            bqkv_t = const.tile([1, 3 * C], F32R)
            nc.sync.dma_start(bqkv_t[:],
                              bqkv.rearrange("(o n) -> o n", o=1).bitcast(F32R))
            bproj_t = const.tile([1, C], BF16)
            nc.sync.dma_start(bproj_t[:],
                              bprojb.rearrange("(o n) -> o n", o=1))
        kron_t = const.tile([16, 1024], BF16)
        nc.sync.dma_start(kron_t[:], kron48[:, :])

        # ---- resident tensors ----
        kp = res.tile([128, CT, N], BF16)           # K^T, LN'd [c, t]
        qnT = res.tile([128, CT, NQ], BF16)         # Q^T, LN'd+scaled [c, t]
        oTr = res.tile([128, CT, NQ], BF16)         # O^T unnormalized
        oTf = res.tile([128, CT, NQ], BF16)         # O^T normalized
        # rows 0-7: softmax row-sums idx 0-7; 32-47: ln scratch
        # (engine operand partition bases must be 32-aligned)
        st48 = res.tile([48, NQ], dt.float32)
        stB = res.tile([8, NQ], dt.float32)         # sums idx 8-15 (aligned base)
        recipsr = res.tile([8, NQ], BF16)           # 1/sums idx 0-7
        recipsB = res.tile([8, NQ], BF16)           # 1/sums idx 8-15
        stmp = res.tile([1, NQ], dt.float32)        # sums-row staging

        # ---- DRAM staging: V-aug per pair ----
        vaug_d = dram.tile([CT, TT, 128, 2, 66], BF16)

        def load_w_group(wq_p, oc_base):
            """[128, CT, C] f32r tile with W_qkv[:, oc_base:oc_base+C].
            One DMA per ic-tile: chunks land on parallel queues and the
            first GEMM starts as soon as chunk 0 arrives."""
            w_t = wq_p.tile([128, CT, C], F32R, tag="w_t")
            wr = wqkv.rearrange("(kt p) c -> p kt c", p=128)
            for kt in range(CT):
                nc.sync.dma_start(w_t[:, kt, :],
                                  wr[:, kt, oc_base:oc_base + C].bitcast(F32R))
            return w_t

        def qkv_psum(ps, x_tile, w_t, oc_base):
            """accumulate x_tile.T @ Wgroup (+ bias) per 512-chunk."""
            for ch in range(2):
                for kt in range(CT):
                    nc.tensor.matmul(
                        ps[:, ts(ch, 512)],
                        x_tile[:, kt, :],
                        w_t[:, kt, ts(ch, 512)],
                        start=(kt == 0),
                        stop=(not with_bias and kt == CT - 1),
                        skip_group_check=True)
                if with_bias:
                    lo = oc_base + ch * 512
                    nc.tensor.matmul(
                        ps[:, ts(ch, 512)], ones1[:],
                        bqkv_t[:, lo:lo + 512],
                        start=False, stop=True, skip_group_check=True)

        def ln_stats(ln_p, ps_tok):
            """DVE bn stats -> mv [128, 2] (mean, var) per token."""
            st6 = ln_p.tile([128, 2, 6], dt.float32, tag="st6")
            nc.vector.bn_stats(st6[:, 0, :], ps_tok[:, 0:512])
            nc.vector.bn_stats(st6[:, 1, :], ps_tok[:, 512:1024])
            mv = ln_p.tile([128, 2], dt.float32, tag="mv")
            nc.vector.bn_aggr(mv[:], st6[:])
            return mv

        def ln_rstd(ln_p, mv, exp_bias):
            """rstd = exp(-0.5*ln(var+eps) + exp_bias) on ACT (ln/exp set;
            exp_bias = ln(scale) tile folds the attention scale for Q).
            General-affine path only: the ln->exp pair thrashes ACT table
            sets (~2.6us/tile), so the trivial path uses ln_rstd_gps."""
            lnv = ln_p.tile([128, 1], dt.float32, tag="lnv")
            nc.scalar.activation(lnv[:], mv[:, 1:2], AF.Ln, bias=eps_t[:])
            rstd = ln_p.tile([128, 1], dt.float32, tag="rstd")
            nc.scalar.activation(rstd[:], lnv[:], AF.Exp, scale=-0.5,
                                 bias=exp_bias)
            return rstd

        def ln_rstd_triv(ln_p, mv, sbias):
            """rstd = scale/sqrt(var+eps) via ACT Sqrt (the only ACT func
            used in P1 -- single table set, no thrash) + exact DVE
            reciprocal (the bass-sanctioned rsqrt pattern). The attention
            scale folds in as sqrt(var/S^2 + eps/S^2) = sqrt(var+eps)/S."""
            sq = ln_p.tile([128, 1], dt.float32, tag="sq")
            nc.scalar.activation(sq[:], mv[:, 1:2], AF.Sqrt,
                                 scale=sbias[0], bias=sbias[1])
            rstd = ln_p.tile([128, 1], dt.float32, tag="rstd")
            nc.vector.reciprocal(rstd[:], sq[:])
            return rstd

        def ln_tail_triv(ln_p, tok_p, pst_p, ps_tok, mv, sbias, sinkT,
                         tslot):
            """deferred LN tail: rstd via ACT Sqrt + DVE reciprocal (single
            ACT table set in P1), normalize on DVE writing bf16 tok,
            grouped bf16 PE transposes, DVE sinks. bn_stats for the tile
            ran right after its GEMM so this tail's tensor_scalar is never
            queued behind a full DVE chain."""
            rstd = ln_rstd_triv(ln_p, mv, sbias)
            tok = tok_p.tile([128, C], BF16, tag="tok")
            nc.vector.tensor_scalar(tok[:], ps_tok[:], mv[:, 0:1], rstd[:],
                                    op0=OP.subtract, op1=OP.mult)
            for g in range(2):
                ps_t = pst_p.tile([128, 512], BF16, tag="ps_t")
                for i in range(4):
                    nc.tensor.matmul(ps_t[:, ts(i, 128)],
                                     tok[:, ts(g * 4 + i, 128)], identb[:],
                                     is_transpose=True, start=True, stop=True,
                                     skip_group_check=True)
                nc.vector.tensor_copy(
                    sinkT[:, g * 4:g * 4 + 4, ts(tslot, 128)],
                    ps_t[:].rearrange("p (i t) -> p i t", i=4))

        def ln_transpose(ln_p, pst_p, ps_tok, exp_bias, g_t, b_t, sink):
            """general affine: per-ct PE transpose + gamma/beta fold in sink."""
            mv = ln_stats(ln_p, ps_tok)
            rstd = ln_rstd(ln_p, mv, exp_bias)
            tok = ln_p.tile([128, C], dt.float32, tag="tok")
            nc.vector.tensor_scalar(tok[:], ps_tok[:], mv[:, 0:1], rstd[:],
                                    op0=OP.subtract, op1=OP.mult)
            for ct in range(CT):
                ps_t = pst_p.tile([128, 128], dt.float32, tag="ps_t")
                nc.tensor.matmul(ps_t[:], tok[:, ts(ct, 128)], ident[:],
                                 is_transpose=True, start=True, stop=True,
                                 skip_group_check=True)
                sink(ct, ps_t, g_t[:, ct:ct + 1], b_t[:, ct:ct + 1])

        def s_exp(ps_s, pt_p, pair, kt, tp, ptag=None):
            """S^T for one head (64 contraction rows at tp) + wide exp."""
            h = tp // 64
            pss = ps_s.tile([128, NQ], dt.float32, tag=ptag or f"pss{h}")
            for qc in range(2):
                nc.tensor.matmul(
                    pss[:, ts(qc, 512)],
                    kp[tp:tp + 64, pair, ts(kt, 128)],
                    qnT[tp:tp + 64, pair, ts(qc, 512)],
                    start=True, stop=True, skip_group_check=True)
            pT = pt_p.tile([128, NQ], BF16, tag=f"pt{h}")
            nc.scalar.activation(pT[:], pss[:], AF.Exp)
            return pT

        def s_pair(ps_s, pt_p, pair, kt):
            """both heads' S^T, h0/h1 interleaved for PE row-group
            concurrency, then one wide exp per head."""
            pss0 = ps_s.tile([128, NQ], dt.float32, tag="pss0")
            pss1 = ps_s.tile([128, NQ], dt.float32, tag="pss1")
            for qc in range(2):
                nc.tensor.matmul(
                    pss0[:, ts(qc, 512)], kp[0:64, pair, ts(kt, 128)],
                    qnT[0:64, pair, ts(qc, 512)],
                    start=True, stop=True, skip_group_check=True)
                nc.tensor.matmul(
                    pss1[:, ts(qc, 512)], kp[64:128, pair, ts(kt, 128)],
                    qnT[64:128, pair, ts(qc, 512)],
                    start=True, stop=True, skip_group_check=True)
            pT0 = pt_p.tile([128, NQ], BF16, tag="pt0")
            nc.scalar.activation(pT0[:], pss0[:], AF.Exp)
            pT1 = pt_p.tile([128, NQ], BF16, tag="pt1")
            nc.scalar.activation(pT1[:], pss1[:], AF.Exp)
            return pT0, pT1

        def pv(po, vsrc, kt, pT):
            for qc in range(2):
                nc.tensor.matmul(
                    po[:, ts(qc, 512)], vsrc,
                    pT[:, ts(qc, 512)],
                    start=(kt == 0), stop=(kt == TT - 1),
                    skip_group_check=True)

        def finish_block(po, pair, h2):
            """save row-sums + unnormalized O^T for (pair, h2)."""
            idx = pair * 2 + h2
            nc.vector.tensor_copy(stmp[:], po[64:65, :])
            if idx < 8:
                nc.sync.dma_start(st48[idx:idx + 1, :], stmp[:])
            else:
                nc.sync.dma_start(stB[idx - 8:idx - 7, :], stmp[:])
            nc.vector.tensor_copy(oTr[h2 * 64:h2 * 64 + 64, pair, :],
                                  po[0:64, :])

        def norm_block(ps_o, pair, h2, rtile):
            """oTf = oTr * bcast(1/sums) for one (pair, h2) block.
            bc reuses the po psum tags (pool is full during attention);
            the resulting WAR chain orders it after the block's finish."""
            idx = pair * 2 + h2
            ridx = idx % 8
            bc = ps_o.tile([66, NQ], dt.float32, tag=f"po{h2}")
            for qc in range(2):
                nc.tensor.matmul(
                    bc[0:64, ts(qc, 512)],
                    kron_t[0:8, ridx * 64:ridx * 64 + 64],
                    rtile[0:8, ts(qc, 512)],
                    start=True, stop=True, skip_group_check=True)
            nc.vector.tensor_tensor(
                oTf[h2 * 64:h2 * 64 + 64, pair, :],
                oTr[h2 * 64:h2 * 64 + 64, pair, :], bc[0:64, :],
                op=OP.mult)

        for _rep in range(reps):
            # ============ P1a: Q group (query half) ============
            with ExitStack() as p1:
                wq_p = p1.enter_context(tc.tile_pool(name="wq", bufs=2))
                xt_p = p1.enter_context(tc.tile_pool(name="xt", bufs=2))
                ln_p = p1.enter_context(tc.tile_pool(name="ln", bufs=3))
                tok_p = p1.enter_context(tc.tile_pool(name="tok", bufs=3))
                ps_p = p1.enter_context(tc.tile_pool(name="ps1", bufs=3,
                                                     space="PSUM"))
                pst_p = p1.enter_context(tc.tile_pool(name="pst", bufs=2,
                                                      space="PSUM"))

                # LN emits are deferred 2 GEMMs back so the stats/rstd/
                # normalize chain (~4.5us latency) never stalls the PE
                # transposes; psum bufs=3 covers the 3 live generations.
                pend = []

                def drain_pend(limit):
                    while len(pend) > limit:
                        pend.pop(0)()

                # first x tile ahead of the weight chunks so the queue
                # doesn't make the first GEMM wait behind all of W
                xt0 = xt_p.tile([128, CT, 128], F32R, tag="xt")
                nc.sync.dma_start(xt0[:].rearrange("p a b -> p (a b)"),
                                  xTqt[0].bitcast(F32R))
                wq = load_w_group(wq_p, 0)
                wk = None
                for tq in range(TQ):
                    if tq == 0:
                        xt = xt0
                    else:
                        xt = xt_p.tile([128, CT, 128], F32R, tag="xt")
                        nc.sync.dma_start(
                            xt[:].rearrange("p a b -> p (a b)"),
                            xTqt[tq].bitcast(F32R))
                    ps_q = ps_p.tile([128, C], dt.float32, tag="ps")
                    qkv_psum(ps_q, xt, wq, 0)
                    if tq == 0:
                        # prefetch K weights behind the Q-phase traffic
                        wk = load_w_group(wq_p, C)

                    def q_sink(ct, ps_t, g, b, tq=tq):
                        nc.vector.tensor_scalar(
                            qnT[:, ct, ts(tq, 128)], ps_t[:], g, b,
                            op0=OP.mult, op1=OP.add)

                    if trivial_affine:
                        mv_q = ln_stats(ln_p, ps_q)

                        def q_emit(ps_q=ps_q, tq=tq, mv_q=mv_q):
                            ln_tail_triv(ln_p, tok_p, pst_p, ps_q, mv_q,
                                         (1.0 / SCALE ** 2, epsq_t[:]),
                                         qnT, tq)
                    else:
                        def q_emit(ps_q=ps_q, tq=tq, q_sink=q_sink):
                            ln_transpose(ln_p, pst_p, ps_q, 0.0,
                                         gq_t, bq_t, q_sink)
                    pend.append(q_emit)
                    drain_pend(2)

                # ============ P1b: K group (full seq) ============
                for tt in range(TT):
                    xt = xt_p.tile([128, CT, 128], F32R, tag="xt")
                    nc.sync.dma_start(
                        xt[:].rearrange("p a b -> p (a b)"),
                        xTt[tt].bitcast(F32R))
                    ps_k = ps_p.tile([128, C], dt.float32, tag="ps")
                    qkv_psum(ps_k, xt, wk, C)

                    def k_sink(ct, ps_t, g, b, tt=tt):
                        nc.vector.tensor_scalar(
                            kp[:, ct, ts(tt, 128)], ps_t[:], g, b,
                            op0=OP.mult, op1=OP.add)

                    if trivial_affine:
                        mv_k = ln_stats(ln_p, ps_k)

                        def k_emit(ps_k=ps_k, tt=tt, mv_k=mv_k):
                            ln_tail_triv(ln_p, tok_p, pst_p, ps_k, mv_k,
                                         (1.0, eps_t[:]), kp, tt)
                    else:
                        def k_emit(ps_k=ps_k, tt=tt, k_sink=k_sink):
                            ln_transpose(ln_p, pst_p, ps_k, 0.0,
                                         gk_t, bk_t, k_sink)
                    pend.append(k_emit)
                    drain_pend(2)
                drain_pend(0)

            # prefetch the projection weights early (2MB; used in P3)
            if _rep == 0:
                wp_p = top.enter_context(tc.tile_pool(name="wp", bufs=1))
            wp = wp_p.tile([128, CT, C], BF16, tag="wp")
            nc.sync.dma_start(wp[:],
                              wprojb.rearrange("(kt p) c -> p kt c", p=128))

            # ==== P2a: V group + fused pair0 (both heads) attention ====
            with ExitStack() as p2a:
                wq_p = p2a.enter_context(tc.tile_pool(name="wv", bufs=1))
                xt_p = p2a.enter_context(tc.tile_pool(name="xt2", bufs=2))
                st_p = p2a.enter_context(tc.tile_pool(name="st", bufs=2))
                pt_p = p2a.enter_context(tc.tile_pool(name="pt", bufs=2))
                ps_p = p2a.enter_context(tc.tile_pool(name="psv", bufs=1,
                                                      space="PSUM"))
                ps_s = p2a.enter_context(tc.tile_pool(name="ps_s", bufs=1,
                                                      space="PSUM"))
                ps_o = p2a.enter_context(tc.tile_pool(name="ps_o", bufs=1,
                                                      space="PSUM"))

                wv = load_w_group(wq_p, 2 * C)
                po0 = ps_o.tile([66, NQ], dt.float32, tag="po0")
                po1 = ps_o.tile([66, NQ], dt.float32, tag="po1")
                prev = pvst = None
                for tt in range(TT):
                    xt = xt_p.tile([128, CT, 128], F32R, tag="xt")
                    nc.sync.dma_start(
                        xt[:].rearrange("p a b -> p (a b)"),
                        xTt[tt].bitcast(F32R))
                    # pair0-h0 S^T first (single shared pss buffer), V GEMM
                    # fills the PE while its exp runs, then pair0-h1 S^T.
                    c0 = s_exp(ps_s, pt_p, 0, tt, 0, ptag="pssA")
                    ps_v = ps_p.tile([128, C], dt.float32, tag="ps")
                    qkv_psum(ps_v, xt, wv, 2 * C)
                    c1 = s_exp(ps_s, pt_p, 0, tt, 64, ptag="pssA")
                    vst = st_p.tile([128, CT, 2, 66], BF16, tag="vst")
                    for half8 in range(2):
                        nc.vector.tensor_copy(
                            vst[:, half8 * 4:half8 * 4 + 4, :, 0:64],
                            ps_v[:, ts(half8, 512)]
                            .rearrange("p (pr b c) -> p pr b c", pr=4, b=2))
                    nc.vector.tensor_copy(
                        vst[:, :, :, 64],
                        ones16[:].rearrange("p (a b) -> p a b", a=8))
                    nc.vector.memset(vst[:, :, :, 65], 0.0)
                    nc.sync.dma_start(
                        vaug_d[:, tt, :, :, :]
                        .rearrange("ct p b c -> p ct b c"), vst[:])

                    if prev is not None:
                        pv(po0, pvst[:, 0, 0, :], tt - 1, prev[0])
                        pv(po1, pvst[:, 0, 1, :], tt - 1, prev[1])
                    prev, pvst = (c0, c1), vst
                pv(po0, pvst[:, 0, 0, :], TT - 1, prev[0])
                pv(po1, pvst[:, 0, 1, :], TT - 1, prev[1])
                finish_block(po0, 0, 0)
                finish_block(po1, 0, 1)

            # ============ P2b: remaining 14 attention blocks ============
            with ExitStack() as p2b:
                vg_p = p2b.enter_context(tc.tile_pool(name="vg", bufs=2))
                pt_p = p2b.enter_context(tc.tile_pool(name="pt2", bufs=3))
                ps_s = p2b.enter_context(tc.tile_pool(name="ps_s2", bufs=1,
                                                      space="PSUM"))
                ps_o = p2b.enter_context(tc.tile_pool(name="ps_o2", bufs=1,
                                                      space="PSUM"))

                def vload(pair):
                    vg = vg_p.tile([128, TT, 2, 66], BF16, tag="vg")
                    nc.sync.dma_start(
                        vg[:], vaug_d[pair, :, :, :, :]
                        .rearrange("tt p b c -> p tt b c"))
                    return vg

                vg = vload(1)
                norm_pend = [(0, 0), (0, 1)]    # pair0 finished in P2a
                for pair in range(1, CT):
                    vg_next = vload(pair + 1) if pair + 1 < CT else None
                    po0 = ps_o.tile([66, NQ], dt.float32, tag="po0")
                    po1 = ps_o.tile([66, NQ], dt.float32, tag="po1")
                    prev = None
                    for kt in range(TT):
                        c0, c1 = s_pair(ps_s, pt_p, pair, kt)
                        if prev is not None:
                            pv(po0, vg[:, kt - 1, 0, :], kt - 1, prev[0])
                            pv(po1, vg[:, kt - 1, 1, :], kt - 1, prev[1])
                        prev = (c0, c1)
                    pv(po0, vg[:, TT - 1, 0, :], TT - 1, prev[0])
                    pv(po1, vg[:, TT - 1, 1, :], TT - 1, prev[1])
                    finish_block(po0, pair, 0)
                    finish_block(po1, pair, 1)
                    norm_pend.append((pair, 0))
                    norm_pend.append((pair, 1))
                    # batch-A reciprocals once sums idx 0-7 (pairs 0-3) ready
                    if pair == 3:
                        # exact DVE reciprocal: off the ACT exp stream and
                        # no ln/exp table switches mid-attention
                        with nc.allow_low_precision(
                                reason="1/sums feeds bf16 rank-1 bcast"):
                            nc.vector.reciprocal(recipsr[:, :],
                                                 st48[0:8, :])
                    elif pair > 3:
                        # drain 2-3 batch-A normalize blocks per pair gap
                        for _ in range(3 if pair > 5 else 2):
                            if norm_pend and norm_pend[0][0] <= 3:
                                p_, h_ = norm_pend.pop(0)
                                norm_block(ps_o, p_, h_, recipsr)
                    vg = vg_next

            # ==== P3: batch-B normalize overlapped with projection ====
            with ExitStack() as p3:
                os_p = p3.enter_context(tc.tile_pool(name="os", bufs=2))
                ps_b = p3.enter_context(tc.tile_pool(name="psb", bufs=1,
                                                     space="PSUM"))
                ps_p3 = p3.enter_context(tc.tile_pool(name="ps3", bufs=1,
                                                      space="PSUM"))

                def proj_mms(ps, tq, oc, ct_lo, ct_hi):
                    for ct in range(ct_lo, ct_hi):
                        nc.tensor.matmul(
                            ps[:], oTf[:, ct, ts(tq, 128)],
                            wp[:, ct, ts(oc, 512)],
                            start=(ct == 0),
                            stop=(not with_bias and ct == CT - 1),
                            skip_group_check=True)
                    if with_bias and ct_hi == CT:
                        nc.tensor.matmul(
                            ps[:], ones1b[:], bproj_t[:, ts(oc, 512)],
                            start=False, stop=True, skip_group_check=True)

                def proj_finish(ps, ost, tq, oc):
                    nc.vector.tensor_copy(ost[:, ts(oc, 512)], ps[:])
                    if oc == 1:
                        nc.sync.dma_start(out[ts(tq, 128), :], ost[:])

                # tq 0-1: pairs-0-3 contraction first, emitted ahead of the
                # batch-B reciprocal/normalize so the PE isn't gated on it
                held = {}
                for tq in range(2):
                    ost = os_p.tile([128, C], dt.float32, tag=f"ost{tq}")
                    for oc in range(2):
                        ps = ps_p3.tile([128, 512], dt.float32,
                                        tag=f"c{tq}{oc}")
                        proj_mms(ps, tq, oc, 0, 4)
                        held[(tq, oc)] = (ps, ost)

                # batch-B reciprocals (pairs 4-7, sums in stB)
                with nc.allow_low_precision(
                        reason="1/sums feeds bf16 rank-1 bcast"):
                    nc.vector.reciprocal(recipsB[:, :], stB[:, :])
                for p_, h_ in norm_pend:
                    norm_block(ps_b, p_, h_,
                               recipsr if p_ * 2 + h_ < 8 else recipsB)

                for tq in range(2):
                    for oc in range(2):
                        ps, ost = held[(tq, oc)]
                        proj_mms(ps, tq, oc, 4, CT)
                        proj_finish(ps, ost, tq, oc)
                for tq in range(2, TQ):
                    ost = os_p.tile([128, C], dt.float32, tag=f"ost{tq % 2}")
                    for oc in range(2):
                        ps = ps_p3.tile([128, 512], dt.float32,
                                        tag=f"c{tq % 2}{oc}")
                        proj_mms(ps, tq, oc, 0, CT)
                        proj_finish(ps, ost, tq, oc)

    nc.compile()
    return nc


_NC = None
_NC_BIAS = None


def _get_nc():
    global _NC
    if _NC is None:
        _NC = build_nc(with_bias=False, trivial_affine=True)
    return _NC


def _get_nc_bias():
    global _NC_BIAS
    if _NC_BIAS is None:
        _NC_BIAS = build_nc(with_bias=True)
    return _NC_BIAS


def _shard_inputs(inputs):
    x = np.asarray(inputs["x"], dtype=np.float32)
    shared = {
        "wqkv": np.asarray(inputs["W_qkv"], dtype=np.float32),
        "wprojb": np.asarray(inputs["W_proj"]).astype(ml_dtypes.bfloat16),
        "bqkv": np.asarray(inputs["b_qkv"], dtype=np.float32),
        "bprojb": np.asarray(inputs["b_proj"]).astype(ml_dtypes.bfloat16),
        "ones128": np.ones(128, dtype=np.float32),
        "kron48": np.kron(np.eye(16, dtype=np.float32),
                          np.ones((1, 64), dtype=np.float32))
        .astype(ml_dtypes.bfloat16),
        "gq": np.asarray(inputs["q_gamma"], dtype=np.float32) * np.float32(SCALE),
        "bq": np.asarray(inputs["q_beta"], dtype=np.float32) * np.float32(SCALE),
        "gk": np.asarray(inputs["k_gamma"], dtype=np.float32),
        "bk": np.asarray(inputs["k_beta"], dtype=np.float32),
    }
    in_maps = []
    for core in range(8):
        b, half = core // 2, core % 2
        # xTt[tt, p, kt*128+j] = x[b].T[kt*128+p, tt*128+j]
        xt4 = x[b].T.reshape(CT, 128, TT, 128)
        xtt = np.ascontiguousarray(xt4.transpose(2, 1, 0, 3).reshape(TT, 128, C))
        m = dict(shared)
        m["xTt"] = xtt
        m["xTqt"] = np.ascontiguousarray(
            xtt[half * TQ:(half + 1) * TQ])
        in_maps.append(m)
    return in_maps


def kernel(**inputs) -> np.ndarray:
    from concourse.bass_utils import run_bass_kernel_spmd
    zero_bias = (not np.any(np.asarray(inputs["b_qkv"]))
                 and not np.any(np.asarray(inputs["b_proj"])))
    trivial = (np.all(np.asarray(inputs["q_gamma"]) == 1)
               and np.all(np.asarray(inputs["k_gamma"]) == 1)
               and not np.any(np.asarray(inputs["q_beta"]))
               and not np.any(np.asarray(inputs["k_beta"])))
    nc = (_get_nc() if zero_bias and trivial
          else build_nc(with_bias=not zero_bias, trivial_affine=trivial))
    in_maps = _shard_inputs(inputs)
    res = run_bass_kernel_spmd(nc, in_maps, core_ids=list(range(8)))
    out = np.empty((B, N, C), dtype=np.float32)
    for core in range(8):
        b, half = core // 2, core % 2
        out[b, half * NQ:(half + 1) * NQ, :] = res.results[core]["out"]
    return out



# revision 35
# speedup vs baseline: 1.0336x; 1.0336x over previous
"""MultiHeadSelfAttention (qk-LayerNorm variant) on 8 TRN2 NeuronCores. v3.

v3 changes (vs v2, 728us):
  * P1 LN chain moved off the critical path: rstd via ACT ln/exp (ACT is
    idle in P1), normalize via ACT Identity with per-partition scale/bias
    writing bf16 tok, and the Q^T/K^T transposes via DMA xbar (bf16)
    straight into residents -- P1 loses all PE transposes, PSUM transpose
    tiles, and DVE sink copies (~1.8us/tile PE stall in v2's trace).
  * kp/qnT now bf16 (halves SBUF + transpose traffic; S matmul numerics
    fine post-LN).
  * K-group weights prefetched during Q phase (wq pool bufs=2).
  * Softmax normalize overlapped into P2b: reciprocal batch A (pairs 0-3)
    after pair 3, blocks normalized in the pair gaps via po-tag reuse;
    batch B at tail. P2c phase gone; P3 entry stall (~15us) gone.

Problem (B=4, N=2048, C=1024, H=16, D=64, fp32):
    qkv = x @ W_qkv + b_qkv ; q,k,v = split(qkv)
    q = LN(q)*scale ; k = LN(k)          (LN over full C)
    attn = softmax(q_h @ k_h^T) per head ; o = attn @ v_h
    out = concat_heads(o) @ W_proj + b_proj

Sharding: core i handles batch i//2 and query-half i%2 (1024 query rows).
Each core computes K/V for the full sequence of its batch. No collectives.

v2 design notes (vs v1):
  * ACT (scalar engine) runs ONLY Exp/Log: softmax exp is the hard floor
    (1 elem/cycle/lane, ~285us/core), so LN stats moved to DVE bn_stats
    and rstd/reciprocal computed as exp(-0.5*ln(var+eps)) / exp(-ln(s))
    on ACT -- all in the natural_log_exp table set, zero table thrash.
  * K^T SBUF-resident (64KB/part); V staged to DRAM as bf16 aug tiles
    [64 v | ones | 0] (66 wide, even for bf16 ISA), reloaded per pair.
  * S^T matmuls fp32r with h0 (rows 0-63) / h1 (rows 64-127) issued
    back-to-back: PE row-group concurrency gives ~2x (measured 147ns/MM).
  * PV + projection in bf16 (safe: post-softmax averaging; numerator and
    denominator share pT rounding).
  * pair0-h0 attention fused into the V-production phase so exp starts
    while V tiles are still being produced.
  * Softmax normalization deferred: unnormalized O^T and row-sums are
    saved per (pair,h2); reciprocals batched on ACT, broadcast via K=1
    rank-1 matmuls, applied with one DVE mult per head before proj.
"""
import numpy as np
import ml_dtypes
from contextlib import ExitStack

import concourse.bass as bass


# NOTE: do NOT override BASS_ACT_ROOT_JSON_PATH to merge the ln/exp table
# sets -- the runtime resolves PSEUDO_LOAD_ACT_FUNC_SET against its own
# stock registry and a reordered json hangs the NEFF (mesh desync).
from concourse import bacc
import concourse.tile as tile
import concourse.mybir as mybir
from concourse.masks import make_identity

dt = mybir.dt
AF = mybir.ActivationFunctionType
OP = mybir.AluOpType
ts = bass.ts

B, N, C = 4, 2048, 1024
H, D = 16, 64
NQ = 1024            # query rows per core
SCALE = D ** -0.5
EPS = 1e-6
TT = N // 128        # 16 token tiles (full seq)
TQ = NQ // 128       # 8 token tiles (query half)
CT = C // 128        # 8 channel tiles (= head pairs)
F32R = dt.float32r
BF16 = dt.bfloat16


def build_nc(reps=1, with_bias=False, trivial_affine=True):
    nc = bacc.Bacc()
    xTt = nc.dram_tensor("xTt", [TT, 128, C], dt.float32, kind="ExternalInput")
    xTqt = nc.dram_tensor("xTqt", [TQ, 128, C], dt.float32, kind="ExternalInput")
    wqkv = nc.dram_tensor("wqkv", [C, 3 * C], dt.float32, kind="ExternalInput")
    wprojb = nc.dram_tensor("wprojb", [C, C], BF16, kind="ExternalInput")
    bqkv = nc.dram_tensor("bqkv", [3 * C], dt.float32, kind="ExternalInput")
    bprojb = nc.dram_tensor("bprojb", [C], BF16, kind="ExternalInput")
    ones128 = nc.dram_tensor("ones128", [128], dt.float32, kind="ExternalInput")
    kron48 = nc.dram_tensor("kron48", [16, 1024], BF16, kind="ExternalInput")
    gq = nc.dram_tensor("gq", [C], dt.float32, kind="ExternalInput")
    bq = nc.dram_tensor("bq", [C], dt.float32, kind="ExternalInput")
    gk = nc.dram_tensor("gk", [C], dt.float32, kind="ExternalInput")
    bk = nc.dram_tensor("bk", [C], dt.float32, kind="ExternalInput")
    out = nc.dram_tensor("out", [NQ, C], dt.float32, kind="ExternalOutput")

    with tile.TileContext(nc) as tc, ExitStack() as top:
        const = top.enter_context(tc.tile_pool(name="const", bufs=1))
        dram = top.enter_context(tc.tile_pool(name="dram", bufs=1, space="DRAM"))
        res = top.enter_context(tc.tile_pool(name="res", bufs=1))

        # ---- constants ----
        ident = const.tile([128, 128], dt.float32)
        make_identity(nc, ident[:])
        identb = const.tile([128, 128], BF16)
        nc.vector.tensor_copy(identb[:], ident[:])
        ones1 = const.tile([1, 128], F32R)
        nc.sync.dma_start(ones1[:], ones128.rearrange("(o n) -> o n", o=1)
                          .bitcast(F32R))
        ones1b = const.tile([1, 128], BF16)
        nc.vector.tensor_copy(ones1b[:], ones1[:].bitcast(dt.float32))
        ones16 = const.tile([128, 16], dt.float32)
        nc.vector.memset(ones16[:], 1.0)
        eps_t = const.tile([128, 1], dt.float32)
        nc.vector.memset(eps_t[:], EPS)
        epsq_t = const.tile([128, 1], dt.float32)
        nc.vector.memset(epsq_t[:], EPS / SCALE ** 2)
        gq_t = const.tile([128, CT], dt.float32)
        bq_t = const.tile([128, CT], dt.float32)
        gk_t = const.tile([128, CT], dt.float32)
        bk_t = const.tile([128, CT], dt.float32)
        for t_, d_ in ((gq_t, gq), (bq_t, bq), (gk_t, gk), (bk_t, bk)):
            nc.sync.dma_start(t_[:], d_.rearrange("(ct p) -> p ct", p=128))
        if with_bias:
            bqkv_t = const.tile([1, 3 * C], F32R)
            nc.sync.dma_start(bqkv_t[:],
                              bqkv.rearrange("(o n) -> o n", o=1).bitcast(F32R))
            bproj_t = const.tile([1, C], BF16)
            nc.sync.dma_start(bproj_t[:],
                              bprojb.rearrange("(o n) -> o n", o=1))
        kron_t = const.tile([16, 1024], BF16)
        nc.sync.dma_start(kron_t[:], kron48[:, :])

        # ---- resident tensors ----
        kp = res.tile([128, CT, N], BF16)           # K^T, LN'd [c, t]
        qnT = res.tile([128, CT, NQ], BF16)         # Q^T, LN'd+scaled [c, t]
        oTr = res.tile([128, CT, NQ], BF16)         # O^T unnormalized
        oTf = res.tile([128, CT, NQ], BF16)         # O^T normalized
        # rows 0-7: softmax row-sums idx 0-7; 32-47: ln scratch
        # (engine operand partition bases must be 32-aligned)
        st48 = res.tile([48, NQ], dt.float32)
        stB = res.tile([8, NQ], dt.float32)         # sums idx 8-15 (aligned base)
        recipsr = res.tile([8, NQ], BF16)           # 1/sums idx 0-7
        recipsB = res.tile([8, NQ], BF16)           # 1/sums idx 8-15
        stmp = res.tile([1, NQ], dt.float32)        # sums-row staging

        # ---- DRAM staging: V-aug per pair ----
        vaug_d = dram.tile([CT, TT, 128, 2, 66], BF16)

        def load_w_group(wq_p, oc_base):
            """[128, CT, C] f32r tile with W_qkv[:, oc_base:oc_base+C].
            One DMA per ic-tile: chunks land on parallel queues and the
            first GEMM starts as soon as chunk 0 arrives."""
            w_t = wq_p.tile([128, CT, C], F32R, tag="w_t")
            wr = wqkv.rearrange("(kt p) c -> p kt c", p=128)
            for kt in range(CT):
                nc.sync.dma_start(w_t[:, kt, :],
                                  wr[:, kt, oc_base:oc_base + C].bitcast(F32R))
            return w_t

        def qkv_psum(ps, x_tile, w_t, oc_base):
            """accumulate x_tile.T @ Wgroup (+ bias) per 512-chunk."""
            for ch in range(2):
                for kt in range(CT):
                    nc.tensor.matmul(
                        ps[:, ts(ch, 512)],
                        x_tile[:, kt, :],
                        w_t[:, kt, ts(ch, 512)],
                        start=(kt == 0),
                        stop=(not with_bias and kt == CT - 1),
                        skip_group_check=True)
                if with_bias:
                    lo = oc_base + ch * 512
                    nc.tensor.matmul(
                        ps[:, ts(ch, 512)], ones1[:],
                        bqkv_t[:, lo:lo + 512],
                        start=False, stop=True, skip_group_check=True)

        def ln_stats(ln_p, ps_tok):
            """DVE bn stats -> mv [128, 2] (mean, var) per token."""
            st6 = ln_p.tile([128, 2, 6], dt.float32, tag="st6")
            nc.vector.bn_stats(st6[:, 0, :], ps_tok[:, 0:512])
            nc.vector.bn_stats(st6[:, 1, :], ps_tok[:, 512:1024])
            mv = ln_p.tile([128, 2], dt.float32, tag="mv")
            nc.vector.bn_aggr(mv[:], st6[:])
            return mv

        def ln_rstd(ln_p, mv, exp_bias):
            """rstd = exp(-0.5*ln(var+eps) + exp_bias) on ACT (ln/exp set;
            exp_bias = ln(scale) tile folds the attention scale for Q).
            General-affine path only: the ln->exp pair thrashes ACT table
            sets (~2.6us/tile), so the trivial path uses ln_rstd_gps."""
            lnv = ln_p.tile([128, 1], dt.float32, tag="lnv")
            nc.scalar.activation(lnv[:], mv[:, 1:2], AF.Ln, bias=eps_t[:])
            rstd = ln_p.tile([128, 1], dt.float32, tag="rstd")
            nc.scalar.activation(rstd[:], lnv[:], AF.Exp, scale=-0.5,
                                 bias=exp_bias)
            return rstd

        def ln_rstd_triv(ln_p, mv, sbias):
            """rstd = scale/sqrt(var+eps) via ACT Sqrt (the only ACT func
            used in P1 -- single table set, no thrash) + exact DVE
            reciprocal (the bass-sanctioned rsqrt pattern). The attention
            scale folds in as sqrt(var/S^2 + eps/S^2) = sqrt(var+eps)/S."""
            sq = ln_p.tile([128, 1], dt.float32, tag="sq")
            nc.scalar.activation(sq[:], mv[:, 1:2], AF.Sqrt,
                                 scale=sbias[0], bias=sbias[1])
            rstd = ln_p.tile([128, 1], dt.float32, tag="rstd")
            nc.vector.reciprocal(rstd[:], sq[:])
            return rstd

        def ln_tail_triv(ln_p, tok_p, pst_p, ps_tok, mv, sbias, sinkT,
                         tslot):
            """deferred LN tail: rstd via ACT Sqrt + DVE reciprocal (single
            ACT table set in P1), normalize on DVE writing bf16 tok,
            grouped bf16 PE transposes, DVE sinks. bn_stats for the tile
            ran right after its GEMM so this tail's tensor_scalar is never
            queued behind a full DVE chain."""
            rstd = ln_rstd_triv(ln_p, mv, sbias)
            tok = tok_p.tile([128, C], BF16, tag="tok")
            nc.vector.tensor_scalar(tok[:], ps_tok[:], mv[:, 0:1], rstd[:],
                                    op0=OP.subtract, op1=OP.mult)
            for g in range(2):
                ps_t = pst_p.tile([128, 512], BF16, tag="ps_t")
                for i in range(4):
                    nc.tensor.matmul(ps_t[:, ts(i, 128)],
                                     tok[:, ts(g * 4 + i, 128)], identb[:],
                                     is_transpose=True, start=True, stop=True,
                                     skip_group_check=True)
                nc.vector.tensor_copy(
                    sinkT[:, g * 4:g * 4 + 4, ts(tslot, 128)],
                    ps_t[:].rearrange("p (i t) -> p i t", i=4))

        def ln_transpose(ln_p, pst_p, ps_tok, exp_bias, g_t, b_t, sink):
            """general affine: per-ct PE transpose + gamma/beta fold in sink."""
            mv = ln_stats(ln_p, ps_tok)
            rstd = ln_rstd(ln_p, mv, exp_bias)
            tok = ln_p.tile([128, C], dt.float32, tag="tok")
            nc.vector.tensor_scalar(tok[:], ps_tok[:], mv[:, 0:1], rstd[:],
                                    op0=OP.subtract, op1=OP.mult)
            for ct in range(CT):
                ps_t = pst_p.tile([128, 128], dt.float32, tag="ps_t")
                nc.tensor.matmul(ps_t[:], tok[:, ts(ct, 128)], ident[:],
                                 is_transpose=True, start=True, stop=True,
                                 skip_group_check=True)
                sink(ct, ps_t, g_t[:, ct:ct + 1], b_t[:, ct:ct + 1])

        def s_exp(ps_s, pt_p, pair, kt, tp, ptag=None):
            """S^T for one head (64 contraction rows at tp) + wide exp."""
            h = tp // 64
            pss = ps_s.tile([128, NQ], dt.float32, tag=ptag or f"pss{h}")
            for qc in range(2):
                nc.tensor.matmul(
                    pss[:, ts(qc, 512)],
                    kp[tp:tp + 64, pair, ts(kt, 128)],
                    qnT[tp:tp + 64, pair, ts(qc, 512)],
                    start=True, stop=True, skip_group_check=True)
            pT = pt_p.tile([128, NQ], BF16, tag=f"pt{h}")
            nc.scalar.activation(pT[:], pss[:], AF.Exp)
            return pT

        def s_pair(ps_s, pt_p, pair, kt):
            """both heads' S^T, h0/h1 interleaved for PE row-group
            concurrency, then one wide exp per head."""
            pss0 = ps_s.tile([128, NQ], dt.float32, tag="pss0")
            pss1 = ps_s.tile([128, NQ], dt.float32, tag="pss1")
            for qc in range(2):
                nc.tensor.matmul(
                    pss0[:, ts(qc, 512)], kp[0:64, pair, ts(kt, 128)],
                    qnT[0:64, pair, ts(qc, 512)],
                    start=True, stop=True, skip_group_check=True)
                nc.tensor.matmul(
                    pss1[:, ts(qc, 512)], kp[64:128, pair, ts(kt, 128)],
                    qnT[64:128, pair, ts(qc, 512)],
                    start=True, stop=True, skip_group_check=True)
            pT0 = pt_p.tile([128, NQ], BF16, tag="pt0")
            nc.scalar.activation(pT0[:], pss0[:], AF.Exp)
            pT1 = pt_p.tile([128, NQ], BF16, tag="pt1")
            nc.scalar.activation(pT1[:], pss1[:], AF.Exp)
            return pT0, pT1

        def pv(po, vsrc, kt, pT):
            for qc in range(2):
                nc.tensor.matmul(
                    po[:, ts(qc, 512)], vsrc,
                    pT[:, ts(qc, 512)],
                    start=(kt == 0), stop=(kt == TT - 1),
                    skip_group_check=True)

        def finish_block(po, pair, h2):
            """save row-sums + unnormalized O^T for (pair, h2)."""
            idx = pair * 2 + h2
            nc.vector.tensor_copy(stmp[:], po[64:65, :])
            if idx < 8:
                nc.sync.dma_start(st48[idx:idx + 1, :], stmp[:])
            else:
                nc.sync.dma_start(stB[idx - 8:idx - 7, :], stmp[:])
            nc.vector.tensor_copy(oTr[h2 * 64:h2 * 64 + 64, pair, :],
                                  po[0:64, :])

        def norm_block(ps_o, pair, h2, rtile):
            """oTf = oTr * bcast(1/sums) for one (pair, h2) block.
            bc reuses the po psum tags (pool is full during attention);
            the resulting WAR chain orders it after the block's finish."""
            idx = pair * 2 + h2
            ridx = idx % 8
            bc = ps_o.tile([66, NQ], dt.float32, tag=f"po{h2}")
            for qc in range(2):
                nc.tensor.matmul(
                    bc[0:64, ts(qc, 512)],
                    kron_t[0:8, ridx * 64:ridx * 64 + 64],
                    rtile[0:8, ts(qc, 512)],
                    start=True, stop=True, skip_group_check=True)
            nc.vector.tensor_tensor(
                oTf[h2 * 64:h2 * 64 + 64, pair, :],
                oTr[h2 * 64:h2 * 64 + 64, pair, :], bc[0:64, :],
                op=OP.mult)

        for _rep in range(reps):
            # ============ P1a: Q group (query half) ============
            with ExitStack() as p1:
                wq_p = p1.enter_context(tc.tile_pool(name="wq", bufs=2))
                xt_p = p1.enter_context(tc.tile_pool(name="xt", bufs=2))
                ln_p = p1.enter_context(tc.tile_pool(name="ln", bufs=3))
                tok_p = p1.enter_context(tc.tile_pool(name="tok", bufs=3))
                ps_p = p1.enter_context(tc.tile_pool(name="ps1", bufs=3,
                                                     space="PSUM"))
                pst_p = p1.enter_context(tc.tile_pool(name="pst", bufs=2,
                                                      space="PSUM"))

                # LN emits are deferred 2 GEMMs back so the stats/rstd/
                # normalize chain (~4.5us latency) never stalls the PE
                # transposes; psum bufs=3 covers the 3 live generations.
                pend = []

                def drain_pend(limit):
                    while len(pend) > limit:
                        pend.pop(0)()

                # first x tile ahead of the weight chunks so the queue
                # doesn't make the first GEMM wait behind all of W
                xt0 = xt_p.tile([128, CT, 128], F32R, tag="xt")
                nc.sync.dma_start(xt0[:].rearrange("p a b -> p (a b)"),
                                  xTqt[0].bitcast(F32R))
                wq = load_w_group(wq_p, 0)
                wk = None
                for tq in range(TQ):
                    if tq == 0:
                        xt = xt0
                    else:
                        xt = xt_p.tile([128, CT, 128], F32R, tag="xt")
                        nc.sync.dma_start(
                            xt[:].rearrange("p a b -> p (a b)"),
                            xTqt[tq].bitcast(F32R))
                    ps_q = ps_p.tile([128, C], dt.float32, tag="ps")
                    qkv_psum(ps_q, xt, wq, 0)
                    if tq == 0:
                        # prefetch K weights behind the Q-phase traffic
                        wk = load_w_group(wq_p, C)

                    def q_sink(ct, ps_t, g, b, tq=tq):
                        nc.vector.tensor_scalar(
                            qnT[:, ct, ts(tq, 128)], ps_t[:], g, b,
                            op0=OP.mult, op1=OP.add)

                    if trivial_affine:
                        mv_q = ln_stats(ln_p, ps_q)

                        def q_emit(ps_q=ps_q, tq=tq, mv_q=mv_q):
                            ln_tail_triv(ln_p, tok_p, pst_p, ps_q, mv_q,
                                         (1.0 / SCALE ** 2, epsq_t[:]),
                                         qnT, tq)
                    else:
                        def q_emit(ps_q=ps_q, tq=tq, q_sink=q_sink):
                            ln_transpose(ln_p, pst_p, ps_q, 0.0,
                                         gq_t, bq_t, q_sink)
                    pend.append(q_emit)
                    drain_pend(2)

                # ============ P1b: K group (full seq) ============
                for tt in range(TT):
                    xt = xt_p.tile([128, CT, 128], F32R, tag="xt")
                    nc.sync.dma_start(
                        xt[:].rearrange("p a b -> p (a b)"),
                        xTt[tt].bitcast(F32R))
                    ps_k = ps_p.tile([128, C], dt.float32, tag="ps")
                    qkv_psum(ps_k, xt, wk, C)

                    def k_sink(ct, ps_t, g, b, tt=tt):
                        nc.vector.tensor_scalar(
                            kp[:, ct, ts(tt, 128)], ps_t[:], g, b,
                            op0=OP.mult, op1=OP.add)

                    if trivial_affine:
                        mv_k = ln_stats(ln_p, ps_k)

                        def k_emit(ps_k=ps_k, tt=tt, mv_k=mv_k):
                            ln_tail_triv(ln_p, tok_p, pst_p, ps_k, mv_k,
                                         (1.0, eps_t[:]), kp, tt)
                    else:
                        def k_emit(ps_k=ps_k, tt=tt, k_sink=k_sink):
                            ln_transpose(ln_p, pst_p, ps_k, 0.0,
                                         gk_t, bk_t, k_sink)
                    pend.append(k_emit)
                    drain_pend(2)
                drain_pend(0)

            # prefetch the projection weights early (2MB; used in P3)
            if _rep == 0:
                wp_p = top.enter_context(tc.tile_pool(name="wp", bufs=1))
            wp = wp_p.tile([128, CT, C], BF16, tag="wp")
            nc.sync.dma_start(wp[:],
                              wprojb.rearrange("(kt p) c -> p kt c", p=128))

            # ============ P2a: V group (GEMM-bound, no fusion) ============
            with ExitStack() as p2a:
                wq_p = p2a.enter_context(tc.tile_pool(name="wv", bufs=1))
                xt_p = p2a.enter_context(tc.tile_pool(name="xt2", bufs=2))
                st_p = p2a.enter_context(tc.tile_pool(name="st", bufs=2))
                ps_p = p2a.enter_context(tc.tile_pool(name="psv", bufs=2,
                                                      space="PSUM"))

                wv = load_w_group(wq_p, 2 * C)
                for tt in range(TT):
                    xt = xt_p.tile([128, CT, 128], F32R, tag="xt")
                    nc.sync.dma_start(
                        xt[:].rearrange("p a b -> p (a b)"),
                        xTt[tt].bitcast(F32R))
                    ps_v = ps_p.tile([128, C], dt.float32, tag="ps")
                    qkv_psum(ps_v, xt, wv, 2 * C)
                    vst = st_p.tile([128, CT, 2, 66], BF16, tag="vst")
                    for half8 in range(2):
                        nc.vector.tensor_copy(
                            vst[:, half8 * 4:half8 * 4 + 4, :, 0:64],
                            ps_v[:, ts(half8, 512)]
                            .rearrange("p (pr b c) -> p pr b c", pr=4, b=2))
                    nc.vector.tensor_copy(
                        vst[:, :, :, 64],
                        ones16[:].rearrange("p (a b) -> p a b", a=8))
                    nc.vector.memset(vst[:, :, :, 65], 0.0)
                    nc.sync.dma_start(
                        vaug_d[:, tt, :, :, :]
                        .rearrange("ct p b c -> p ct b c"), vst[:])

            # ============ P2b: all 16 attention blocks ============
            with ExitStack() as p2b:
                vg_p = p2b.enter_context(tc.tile_pool(name="vg", bufs=2))
                pt_p = p2b.enter_context(tc.tile_pool(name="pt2", bufs=3))
                ps_s = p2b.enter_context(tc.tile_pool(name="ps_s2", bufs=1,
                                                      space="PSUM"))
                ps_o = p2b.enter_context(tc.tile_pool(name="ps_o2", bufs=1,
                                                      space="PSUM"))

                def vload(pair):
                    vg = vg_p.tile([128, TT, 2, 66], BF16, tag="vg")
                    nc.sync.dma_start(
                        vg[:], vaug_d[pair, :, :, :, :]
                        .rearrange("tt p b c -> p tt b c"))
                    return vg

                vg = vload(0)
                norm_pend = []
                for pair in range(CT):
                    vg_next = vload(pair + 1) if pair + 1 < CT else None
                    po0 = ps_o.tile([66, NQ], dt.float32, tag="po0")
                    po1 = ps_o.tile([66, NQ], dt.float32, tag="po1")
                    prev = None
                    for kt in range(TT):
                        c0, c1 = s_pair(ps_s, pt_p, pair, kt)
                        if prev is not None:
                            pv(po0, vg[:, kt - 1, 0, :], kt - 1, prev[0])
                            pv(po1, vg[:, kt - 1, 1, :], kt - 1, prev[1])
                        prev = (c0, c1)
                    pv(po0, vg[:, TT - 1, 0, :], TT - 1, prev[0])
                    pv(po1, vg[:, TT - 1, 1, :], TT - 1, prev[1])
                    finish_block(po0, pair, 0)
                    finish_block(po1, pair, 1)
                    norm_pend.append((pair, 0))
                    norm_pend.append((pair, 1))
                    # batch-A reciprocals once sums idx 0-7 (pairs 0-3) ready
                    if pair == 3:
                        # exact DVE reciprocal: off the ACT exp stream and
                        # no ln/exp table switches mid-attention
                        with nc.allow_low_precision(
                                reason="1/sums feeds bf16 rank-1 bcast"):
                            nc.vector.reciprocal(recipsr[:, :],
                                                 st48[0:8, :])
                    elif pair > 3:
                        # drain 2-3 batch-A normalize blocks per pair gap
                        for _ in range(3 if pair > 5 else 2):
                            if norm_pend and norm_pend[0][0] <= 3:
                                p_, h_ = norm_pend.pop(0)
                                norm_block(ps_o, p_, h_, recipsr)
                    vg = vg_next

            # ==== P3: batch-B normalize overlapped with projection ====
            with ExitStack() as p3:
                os_p = p3.enter_context(tc.tile_pool(name="os", bufs=2))
                ps_b = p3.enter_context(tc.tile_pool(name="psb", bufs=1,
                                                     space="PSUM"))
                ps_p3 = p3.enter_context(tc.tile_pool(name="ps3", bufs=1,
                                                      space="PSUM"))

                def proj_mms(ps, tq, oc, ct_lo, ct_hi):
                    for ct in range(ct_lo, ct_hi):
                        nc.tensor.matmul(
                            ps[:], oTf[:, ct, ts(tq, 128)],
                            wp[:, ct, ts(oc, 512)],
                            start=(ct == 0),
                            stop=(not with_bias and ct == CT - 1),
                            skip_group_check=True)
                    if with_bias and ct_hi == CT:
                        nc.tensor.matmul(
                            ps[:], ones1b[:], bproj_t[:, ts(oc, 512)],
                            start=False, stop=True, skip_group_check=True)

                def proj_finish(ps, ost, tq, oc):
                    nc.vector.tensor_copy(ost[:, ts(oc, 512)], ps[:])
                    if oc == 1:
                        nc.sync.dma_start(out[ts(tq, 128), :], ost[:])

                # tq 0-1: pairs-0-3 contraction first, emitted ahead of the
                # batch-B reciprocal/normalize so the PE isn't gated on it
                held = {}
                for tq in range(2):
                    ost = os_p.tile([128, C], dt.float32, tag=f"ost{tq}")
                    for oc in range(2):
                        ps = ps_p3.tile([128, 512], dt.float32,
                                        tag=f"c{tq}{oc}")
                        proj_mms(ps, tq, oc, 0, 4)
                        held[(tq, oc)] = (ps, ost)

                # batch-B reciprocals (pairs 4-7, sums in stB)
                with nc.allow_low_precision(
                        reason="1/sums feeds bf16 rank-1 bcast"):
                    nc.vector.reciprocal(recipsB[:, :], stB[:, :])
                for p_, h_ in norm_pend:
                    norm_block(ps_b, p_, h_,
                               recipsr if p_ * 2 + h_ < 8 else recipsB)

                for tq in range(2):
                    for oc in range(2):
                        ps, ost = held[(tq, oc)]
                        proj_mms(ps, tq, oc, 4, CT)
                        proj_finish(ps, ost, tq, oc)
                for tq in range(2, TQ):
                    ost = os_p.tile([128, C], dt.float32, tag=f"ost{tq % 2}")
                    for oc in range(2):
                        ps = ps_p3.tile([128, 512], dt.float32,
                                        tag=f"c{tq % 2}{oc}")
                        proj_mms(ps, tq, oc, 0, CT)
                        proj_finish(ps, ost, tq, oc)

    nc.compile()
    return nc


_NC = None
_NC_BIAS = None


def _get_nc():
    global _NC
    if _NC is None:
        _NC = build_nc(with_bias=False, trivial_affine=True)
    return _NC


def _get_nc_bias():
    global _NC_BIAS
    if _NC_BIAS is None:
        _NC_BIAS = build_nc(with_bias=True)
    return _NC_BIAS


def _shard_inputs(inputs):
    x = np.asarray(inputs["x"], dtype=np.float32)
    shared = {
        "wqkv": np.asarray(inputs["W_qkv"], dtype=np.float32),
        "wprojb": np.asarray(inputs["W_proj"]).astype(ml_dtypes.bfloat16),
        "bqkv": np.asarray(inputs["b_qkv"], dtype=np.float32),
        "bprojb": np.asarray(inputs["b_proj"]).astype(ml_dtypes.bfloat16),
        "ones128": np.ones(128, dtype=np.float32),
        "kron48": np.kron(np.eye(16, dtype=np.float32),
                          np.ones((1, 64), dtype=np.float32))
        .astype(ml_dtypes.bfloat16),
        "gq": np.asarray(inputs["q_gamma"], dtype=np.float32) * np.float32(SCALE),
        "bq": np.asarray(inputs["q_beta"], dtype=np.float32) * np.float32(SCALE),
        "gk": np.asarray(inputs["k_gamma"], dtype=np.float32),
        "bk": np.asarray(inputs["k_beta"], dtype=np.float32),
    }
    in_maps = []
    for core in range(8):
        b, half = core // 2, core % 2
        # xTt[tt, p, kt*128+j] = x[b].T[kt*128+p, tt*128+j]
        xt4 = x[b].T.reshape(CT, 128, TT, 128)
        xtt = np.ascontiguousarray(xt4.transpose(2, 1, 0, 3).reshape(TT, 128, C))
        m = dict(shared)
        m["xTt"] = xtt
        m["xTqt"] = np.ascontiguousarray(
            xtt[half * TQ:(half + 1) * TQ])
        in_maps.append(m)
    return in_maps


def kernel(**inputs) -> np.ndarray:
    from concourse.bass_utils import run_bass_kernel_spmd
    zero_bias = (not np.any(np.asarray(inputs["b_qkv"]))
                 and not np.any(np.asarray(inputs["b_proj"])))
    trivial = (np.all(np.asarray(inputs["q_gamma"]) == 1)
               and np.all(np.asarray(inputs["k_gamma"]) == 1)
               and not np.any(np.asarray(inputs["q_beta"]))
               and not np.any(np.asarray(inputs["k_beta"])))
    nc = (_get_nc() if zero_bias and trivial
          else build_nc(with_bias=not zero_bias, trivial_affine=trivial))
    in_maps = _shard_inputs(inputs)
    res = run_bass_kernel_spmd(nc, in_maps, core_ids=list(range(8)))
    out = np.empty((B, N, C), dtype=np.float32)
    for core in range(8):
        b, half = core // 2, core % 2
        out[b, half * NQ:(half + 1) * NQ, :] = res.results[core]["out"]
    return out



# revision 36
# speedup vs baseline: 1.0900x; 1.0546x over previous
"""MultiHeadSelfAttention (qk-LayerNorm variant) on 8 TRN2 NeuronCores. v3.

v3 changes (vs v2, 728us):
  * P1 LN chain moved off the critical path: rstd via ACT ln/exp (ACT is
    idle in P1), normalize via ACT Identity with per-partition scale/bias
    writing bf16 tok, and the Q^T/K^T transposes via DMA xbar (bf16)
    straight into residents -- P1 loses all PE transposes, PSUM transpose
    tiles, and DVE sink copies (~1.8us/tile PE stall in v2's trace).
  * kp/qnT now bf16 (halves SBUF + transpose traffic; S matmul numerics
    fine post-LN).
  * K-group weights prefetched during Q phase (wq pool bufs=2).
  * Softmax normalize overlapped into P2b: reciprocal batch A (pairs 0-3)
    after pair 3, blocks normalized in the pair gaps via po-tag reuse;
    batch B at tail. P2c phase gone; P3 entry stall (~15us) gone.

Problem (B=4, N=2048, C=1024, H=16, D=64, fp32):
    qkv = x @ W_qkv + b_qkv ; q,k,v = split(qkv)
    q = LN(q)*scale ; k = LN(k)          (LN over full C)
    attn = softmax(q_h @ k_h^T) per head ; o = attn @ v_h
    out = concat_heads(o) @ W_proj + b_proj

Sharding: core i handles batch i//2 and query-half i%2 (1024 query rows).
Each core computes K/V for the full sequence of its batch. No collectives.

v2 design notes (vs v1):
  * ACT (scalar engine) runs ONLY Exp/Log: softmax exp is the hard floor
    (1 elem/cycle/lane, ~285us/core), so LN stats moved to DVE bn_stats
    and rstd/reciprocal computed as exp(-0.5*ln(var+eps)) / exp(-ln(s))
    on ACT -- all in the natural_log_exp table set, zero table thrash.
  * K^T SBUF-resident (64KB/part); V staged to DRAM as bf16 aug tiles
    [64 v | ones | 0] (66 wide, even for bf16 ISA), reloaded per pair.
  * S^T matmuls fp32r with h0 (rows 0-63) / h1 (rows 64-127) issued
    back-to-back: PE row-group concurrency gives ~2x (measured 147ns/MM).
  * PV + projection in bf16 (safe: post-softmax averaging; numerator and
    denominator share pT rounding).
  * pair0-h0 attention fused into the V-production phase so exp starts
    while V tiles are still being produced.
  * Softmax normalization deferred: unnormalized O^T and row-sums are
    saved per (pair,h2); reciprocals batched on ACT, broadcast via K=1
    rank-1 matmuls, applied with one DVE mult per head before proj.
"""
import numpy as np
import ml_dtypes
from contextlib import ExitStack

import concourse.bass as bass


# NOTE: do NOT override BASS_ACT_ROOT_JSON_PATH to merge the ln/exp table
# sets -- the runtime resolves PSEUDO_LOAD_ACT_FUNC_SET against its own
# stock registry and a reordered json hangs the NEFF (mesh desync).
from concourse import bacc
import concourse.tile as tile
import concourse.mybir as mybir
from concourse.masks import make_identity

dt = mybir.dt
AF = mybir.ActivationFunctionType
OP = mybir.AluOpType
ts = bass.ts

B, N, C = 4, 2048, 1024
H, D = 16, 64
NQ = 1024            # query rows per core
SCALE = D ** -0.5
EPS = 1e-6
TT = N // 128        # 16 token tiles (full seq)
TQ = NQ // 128       # 8 token tiles (query half)
CT = C // 128        # 8 channel tiles (= head pairs)
F32R = dt.float32r
BF16 = dt.bfloat16


def build_nc(reps=1, with_bias=False, trivial_affine=True):
    nc = bacc.Bacc()
    xTt = nc.dram_tensor("xTt", [TT, 128, C], BF16, kind="ExternalInput")
    xTqt = nc.dram_tensor("xTqt", [TQ, 128, C], BF16, kind="ExternalInput")
    wqkv = nc.dram_tensor("wqkv", [C, 3 * C], BF16, kind="ExternalInput")
    wprojb = nc.dram_tensor("wprojb", [C, C], BF16, kind="ExternalInput")
    bqkv = nc.dram_tensor("bqkv", [3 * C], dt.float32, kind="ExternalInput")
    bprojb = nc.dram_tensor("bprojb", [C], BF16, kind="ExternalInput")
    ones128 = nc.dram_tensor("ones128", [128], dt.float32, kind="ExternalInput")
    kron48 = nc.dram_tensor("kron48", [16, 1024], BF16, kind="ExternalInput")
    gq = nc.dram_tensor("gq", [C], dt.float32, kind="ExternalInput")
    bq = nc.dram_tensor("bq", [C], dt.float32, kind="ExternalInput")
    gk = nc.dram_tensor("gk", [C], dt.float32, kind="ExternalInput")
    bk = nc.dram_tensor("bk", [C], dt.float32, kind="ExternalInput")
    out = nc.dram_tensor("out", [NQ, C], dt.float32, kind="ExternalOutput")

    with tile.TileContext(nc) as tc, ExitStack() as top:
        const = top.enter_context(tc.tile_pool(name="const", bufs=1))
        dram = top.enter_context(tc.tile_pool(name="dram", bufs=1, space="DRAM"))
        res = top.enter_context(tc.tile_pool(name="res", bufs=1))

        # ---- constants ----
        ident = const.tile([128, 128], dt.float32)
        make_identity(nc, ident[:])
        identb = const.tile([128, 128], BF16)
        nc.vector.tensor_copy(identb[:], ident[:])
        ones1 = const.tile([1, 128], F32R)
        nc.sync.dma_start(ones1[:], ones128.rearrange("(o n) -> o n", o=1)
                          .bitcast(F32R))
        ones1b = const.tile([1, 128], BF16)
        nc.vector.tensor_copy(ones1b[:], ones1[:].bitcast(dt.float32))
        ones16 = const.tile([128, 16], dt.float32)
        nc.vector.memset(ones16[:], 1.0)
        eps_t = const.tile([128, 1], dt.float32)
        nc.vector.memset(eps_t[:], EPS)
        epsq_t = const.tile([128, 1], dt.float32)
        nc.vector.memset(epsq_t[:], EPS / SCALE ** 2)
        gq_t = const.tile([128, CT], dt.float32)
        bq_t = const.tile([128, CT], dt.float32)
        gk_t = const.tile([128, CT], dt.float32)
        bk_t = const.tile([128, CT], dt.float32)
        for t_, d_ in ((gq_t, gq), (bq_t, bq), (gk_t, gk), (bk_t, bk)):
            nc.sync.dma_start(t_[:], d_.rearrange("(ct p) -> p ct", p=128))
        if with_bias:
            bqkv_t = const.tile([1, 3 * C], F32R)
            nc.sync.dma_start(bqkv_t[:],
                              bqkv.rearrange("(o n) -> o n", o=1).bitcast(F32R))
            bproj_t = const.tile([1, C], BF16)
            nc.sync.dma_start(bproj_t[:],
                              bprojb.rearrange("(o n) -> o n", o=1))
        kron_t = const.tile([16, 1024], BF16)
        nc.sync.dma_start(kron_t[:], kron48[:, :])

        # ---- resident tensors ----
        kp = res.tile([128, CT, N], BF16)           # K^T, LN'd [c, t]
        qnT = res.tile([128, CT, NQ], BF16)         # Q^T, LN'd+scaled [c, t]
        oTr = res.tile([128, CT, NQ], BF16)         # O^T unnormalized
        oTf = res.tile([128, CT, NQ], BF16)         # O^T normalized
        # rows 0-7: softmax row-sums idx 0-7; 32-47: ln scratch
        # (engine operand partition bases must be 32-aligned)
        st48 = res.tile([48, NQ], dt.float32)
        stB = res.tile([8, NQ], dt.float32)         # sums idx 8-15 (aligned base)
        recipsr = res.tile([8, NQ], BF16)           # 1/sums idx 0-7
        recipsB = res.tile([8, NQ], BF16)           # 1/sums idx 8-15
        stmp = res.tile([1, NQ], dt.float32)        # sums-row staging

        # ---- DRAM staging: V-aug per pair ----
        vaug_d = dram.tile([CT, TT, 128, 2, 66], BF16)

        def load_w_group(wq_p, oc_base):
            """[128, CT, C] f32r tile with W_qkv[:, oc_base:oc_base+C].
            One DMA per ic-tile: chunks land on parallel queues and the
            first GEMM starts as soon as chunk 0 arrives."""
            w_t = wq_p.tile([128, CT, C], BF16, tag="w_t")
            wr = wqkv.rearrange("(kt p) c -> p kt c", p=128)
            for kt in range(CT):
                nc.sync.dma_start(w_t[:, kt, :],
                                  wr[:, kt, oc_base:oc_base + C])
            return w_t

        def qkv_psum(ps, x_tile, w_t, oc_base):
            """accumulate x_tile.T @ Wgroup (+ bias) per 512-chunk."""
            for ch in range(2):
                for kt in range(CT):
                    nc.tensor.matmul(
                        ps[:, ts(ch, 512)],
                        x_tile[:, kt, :],
                        w_t[:, kt, ts(ch, 512)],
                        start=(kt == 0),
                        stop=(not with_bias and kt == CT - 1),
                        skip_group_check=True)
                if with_bias:
                    lo = oc_base + ch * 512
                    nc.tensor.matmul(
                        ps[:, ts(ch, 512)], ones1[:],
                        bqkv_t[:, lo:lo + 512],
                        start=False, stop=True, skip_group_check=True)

        def ln_stats(ln_p, ps_tok):
            """DVE bn stats -> mv [128, 2] (mean, var) per token."""
            st6 = ln_p.tile([128, 2, 6], dt.float32, tag="st6")
            nc.vector.bn_stats(st6[:, 0, :], ps_tok[:, 0:512])
            nc.vector.bn_stats(st6[:, 1, :], ps_tok[:, 512:1024])
            mv = ln_p.tile([128, 2], dt.float32, tag="mv")
            nc.vector.bn_aggr(mv[:], st6[:])
            return mv

        def ln_rstd(ln_p, mv, exp_bias):
            """rstd = exp(-0.5*ln(var+eps) + exp_bias) on ACT (ln/exp set;
            exp_bias = ln(scale) tile folds the attention scale for Q).
            General-affine path only: the ln->exp pair thrashes ACT table
            sets (~2.6us/tile), so the trivial path uses ln_rstd_gps."""
            lnv = ln_p.tile([128, 1], dt.float32, tag="lnv")
            nc.scalar.activation(lnv[:], mv[:, 1:2], AF.Ln, bias=eps_t[:])
            rstd = ln_p.tile([128, 1], dt.float32, tag="rstd")
            nc.scalar.activation(rstd[:], lnv[:], AF.Exp, scale=-0.5,
                                 bias=exp_bias)
            return rstd

        def ln_rstd_triv(ln_p, mv, sbias):
            """rstd = scale/sqrt(var+eps) via ACT Sqrt (the only ACT func
            used in P1 -- single table set, no thrash) + exact DVE
            reciprocal (the bass-sanctioned rsqrt pattern). The attention
            scale folds in as sqrt(var/S^2 + eps/S^2) = sqrt(var+eps)/S."""
            sq = ln_p.tile([128, 1], dt.float32, tag="sq")
            nc.scalar.activation(sq[:], mv[:, 1:2], AF.Sqrt,
                                 scale=sbias[0], bias=sbias[1])
            rstd = ln_p.tile([128, 1], dt.float32, tag="rstd")
            nc.vector.reciprocal(rstd[:], sq[:])
            return rstd

        def ln_tail_triv(ln_p, tok_p, pst_p, ps_tok, mv, sbias, sinkT,
                         tslot):
            """deferred LN tail: rstd via ACT Sqrt + DVE reciprocal (single
            ACT table set in P1), normalize on DVE writing bf16 tok,
            grouped bf16 PE transposes, DVE sinks. bn_stats for the tile
            ran right after its GEMM so this tail's tensor_scalar is never
            queued behind a full DVE chain."""
            rstd = ln_rstd_triv(ln_p, mv, sbias)
            tok = tok_p.tile([128, C], BF16, tag="tok")
            nc.vector.tensor_scalar(tok[:], ps_tok[:], mv[:, 0:1], rstd[:],
                                    op0=OP.subtract, op1=OP.mult)
            for g in range(2):
                ps_t = pst_p.tile([128, 512], BF16, tag="ps_t")
                for i in range(4):
                    nc.tensor.matmul(ps_t[:, ts(i, 128)],
                                     tok[:, ts(g * 4 + i, 128)], identb[:],
                                     is_transpose=True, start=True, stop=True,
                                     skip_group_check=True)
                nc.vector.tensor_copy(
                    sinkT[:, g * 4:g * 4 + 4, ts(tslot, 128)],
                    ps_t[:].rearrange("p (i t) -> p i t", i=4))

        def ln_transpose(ln_p, pst_p, ps_tok, exp_bias, g_t, b_t, sink):
            """general affine: per-ct PE transpose + gamma/beta fold in sink."""
            mv = ln_stats(ln_p, ps_tok)
            rstd = ln_rstd(ln_p, mv, exp_bias)
            tok = ln_p.tile([128, C], dt.float32, tag="tok")
            nc.vector.tensor_scalar(tok[:], ps_tok[:], mv[:, 0:1], rstd[:],
                                    op0=OP.subtract, op1=OP.mult)
            for ct in range(CT):
                ps_t = pst_p.tile([128, 128], dt.float32, tag="ps_t")
                nc.tensor.matmul(ps_t[:], tok[:, ts(ct, 128)], ident[:],
                                 is_transpose=True, start=True, stop=True,
                                 skip_group_check=True)
                sink(ct, ps_t, g_t[:, ct:ct + 1], b_t[:, ct:ct + 1])

        def s_exp(ps_s, pt_p, pair, kt, tp, ptag=None):
            """S^T for one head (64 contraction rows at tp) + wide exp."""
            h = tp // 64
            pss = ps_s.tile([128, NQ], dt.float32, tag=ptag or f"pss{h}")
            for qc in range(2):
                nc.tensor.matmul(
                    pss[:, ts(qc, 512)],
                    kp[tp:tp + 64, pair, ts(kt, 128)],
                    qnT[tp:tp + 64, pair, ts(qc, 512)],
                    start=True, stop=True, skip_group_check=True)
            pT = pt_p.tile([128, NQ], BF16, tag=f"pt{h}")
            nc.scalar.activation(pT[:], pss[:], AF.Exp)
            return pT

        def s_pair(ps_s, pt_p, pair, kt):
            """both heads' S^T, h0/h1 interleaved for PE row-group
            concurrency, then one wide exp per head."""
            pss0 = ps_s.tile([128, NQ], dt.float32, tag="pss0")
            pss1 = ps_s.tile([128, NQ], dt.float32, tag="pss1")
            for qc in range(2):
                nc.tensor.matmul(
                    pss0[:, ts(qc, 512)], kp[0:64, pair, ts(kt, 128)],
                    qnT[0:64, pair, ts(qc, 512)],
                    start=True, stop=True, skip_group_check=True)
                nc.tensor.matmul(
                    pss1[:, ts(qc, 512)], kp[64:128, pair, ts(kt, 128)],
                    qnT[64:128, pair, ts(qc, 512)],
                    start=True, stop=True, skip_group_check=True)
            pT0 = pt_p.tile([128, NQ], BF16, tag="pt0")
            nc.scalar.activation(pT0[:], pss0[:], AF.Exp)
            pT1 = pt_p.tile([128, NQ], BF16, tag="pt1")
            nc.scalar.activation(pT1[:], pss1[:], AF.Exp)
            return pT0, pT1

        def pv(po, vsrc, kt, pT):
            for qc in range(2):
                nc.tensor.matmul(
                    po[:, ts(qc, 512)], vsrc,
                    pT[:, ts(qc, 512)],
                    start=(kt == 0), stop=(kt == TT - 1),
                    skip_group_check=True)

        def finish_block(po, pair, h2):
            """save row-sums + unnormalized O^T for (pair, h2)."""
            idx = pair * 2 + h2
            nc.vector.tensor_copy(stmp[:], po[64:65, :])
            if idx < 8:
                nc.sync.dma_start(st48[idx:idx + 1, :], stmp[:])
            else:
                nc.sync.dma_start(stB[idx - 8:idx - 7, :], stmp[:])
            nc.vector.tensor_copy(oTr[h2 * 64:h2 * 64 + 64, pair, :],
                                  po[0:64, :])

        def norm_block(ps_o, pair, h2, rtile):
            """oTf = oTr * bcast(1/sums) for one (pair, h2) block.
            bc reuses the po psum tags (pool is full during attention);
            the resulting WAR chain orders it after the block's finish."""
            idx = pair * 2 + h2
            ridx = idx % 8
            bc = ps_o.tile([66, NQ], dt.float32, tag=f"po{h2}")
            for qc in range(2):
                nc.tensor.matmul(
                    bc[0:64, ts(qc, 512)],
                    kron_t[0:8, ridx * 64:ridx * 64 + 64],
                    rtile[0:8, ts(qc, 512)],
                    start=True, stop=True, skip_group_check=True)
            nc.vector.tensor_tensor(
                oTf[h2 * 64:h2 * 64 + 64, pair, :],
                oTr[h2 * 64:h2 * 64 + 64, pair, :], bc[0:64, :],
                op=OP.mult)

        for _rep in range(reps):
            # ============ P1a: Q group (query half) ============
            if _rep == 0:
                wq_p = top.enter_context(tc.tile_pool(name="wq", bufs=2))
            with ExitStack() as p1:
                xt_p = p1.enter_context(tc.tile_pool(name="xt", bufs=2))
                ln_p = p1.enter_context(tc.tile_pool(name="ln", bufs=3))
                tok_p = p1.enter_context(tc.tile_pool(name="tok", bufs=3))
                ps_p = p1.enter_context(tc.tile_pool(name="ps1", bufs=3,
                                                     space="PSUM"))
                pst_p = p1.enter_context(tc.tile_pool(name="pst", bufs=2,
                                                      space="PSUM"))

                # LN emits are deferred 2 GEMMs back so the stats/rstd/
                # normalize chain (~4.5us latency) never stalls the PE
                # transposes; psum bufs=3 covers the 3 live generations.
                pend = []

                def drain_pend(limit):
                    while len(pend) > limit:
                        pend.pop(0)()

                # first x tile ahead of the weight chunks so the queue
                # doesn't make the first GEMM wait behind all of W
                xt0 = xt_p.tile([128, CT, 128], BF16, tag="xt")
                nc.sync.dma_start(xt0[:].rearrange("p a b -> p (a b)"),
                                  xTqt[0])
                wq = load_w_group(wq_p, 0)
                wk = None
                for tq in range(TQ):
                    if tq == 0:
                        xt = xt0
                    else:
                        xt = xt_p.tile([128, CT, 128], BF16, tag="xt")
                        nc.sync.dma_start(
                            xt[:].rearrange("p a b -> p (a b)"),
                            xTqt[tq])
                    ps_q = ps_p.tile([128, C], dt.float32, tag="ps")
                    qkv_psum(ps_q, xt, wq, 0)
                    if tq == 0:
                        # prefetch K weights behind the Q-phase traffic
                        wk = load_w_group(wq_p, C)

                    def q_sink(ct, ps_t, g, b, tq=tq):
                        nc.vector.tensor_scalar(
                            qnT[:, ct, ts(tq, 128)], ps_t[:], g, b,
                            op0=OP.mult, op1=OP.add)

                    if trivial_affine:
                        mv_q = ln_stats(ln_p, ps_q)

                        def q_emit(ps_q=ps_q, tq=tq, mv_q=mv_q):
                            ln_tail_triv(ln_p, tok_p, pst_p, ps_q, mv_q,
                                         (1.0 / SCALE ** 2, epsq_t[:]),
                                         qnT, tq)
                    else:
                        def q_emit(ps_q=ps_q, tq=tq, q_sink=q_sink):
                            ln_transpose(ln_p, pst_p, ps_q, 0.0,
                                         gq_t, bq_t, q_sink)
                    pend.append(q_emit)
                    drain_pend(2)

                # prefetch V weights as the 3rd w_t generation: the WAR on
                # wq's buffer makes the DMA start as Q-phase ends, landing
                # well inside the K phase
                wv = load_w_group(wq_p, 2 * C)

                # ============ P1b: K group (full seq) ============
                for tt in range(TT):
                    xt = xt_p.tile([128, CT, 128], BF16, tag="xt")
                    nc.sync.dma_start(
                        xt[:].rearrange("p a b -> p (a b)"),
                        xTt[tt])
                    ps_k = ps_p.tile([128, C], dt.float32, tag="ps")
                    qkv_psum(ps_k, xt, wk, C)

                    def k_sink(ct, ps_t, g, b, tt=tt):
                        nc.vector.tensor_scalar(
                            kp[:, ct, ts(tt, 128)], ps_t[:], g, b,
                            op0=OP.mult, op1=OP.add)

                    if trivial_affine:
                        mv_k = ln_stats(ln_p, ps_k)

                        def k_emit(ps_k=ps_k, tt=tt, mv_k=mv_k):
                            ln_tail_triv(ln_p, tok_p, pst_p, ps_k, mv_k,
                                         (1.0, eps_t[:]), kp, tt)
                    else:
                        def k_emit(ps_k=ps_k, tt=tt, k_sink=k_sink):
                            ln_transpose(ln_p, pst_p, ps_k, 0.0,
                                         gk_t, bk_t, k_sink)
                    pend.append(k_emit)
                    drain_pend(2)
                drain_pend(0)

            # prefetch the projection weights early (2MB; used in P3)
            if _rep == 0:
                wp_p = top.enter_context(tc.tile_pool(name="wp", bufs=1))
            wp = wp_p.tile([128, CT, C], BF16, tag="wp")
            nc.sync.dma_start(wp[:],
                              wprojb.rearrange("(kt p) c -> p kt c", p=128))

            # ============ P2a: V group (GEMM-bound, no fusion) ============
            with ExitStack() as p2a:
                xt_p = p2a.enter_context(tc.tile_pool(name="xt2", bufs=2))
                st_p = p2a.enter_context(tc.tile_pool(name="st", bufs=2))
                ps_p = p2a.enter_context(tc.tile_pool(name="psv", bufs=2,
                                                      space="PSUM"))

                for tt in range(TT):
                    xt = xt_p.tile([128, CT, 128], BF16, tag="xt")
                    nc.sync.dma_start(
                        xt[:].rearrange("p a b -> p (a b)"),
                        xTt[tt])
                    ps_v = ps_p.tile([128, C], dt.float32, tag="ps")
                    qkv_psum(ps_v, xt, wv, 2 * C)
                    vst = st_p.tile([128, CT, 2, 66], BF16, tag="vst")
                    for half8 in range(2):
                        nc.vector.tensor_copy(
                            vst[:, half8 * 4:half8 * 4 + 4, :, 0:64],
                            ps_v[:, ts(half8, 512)]
                            .rearrange("p (pr b c) -> p pr b c", pr=4, b=2))
                    nc.vector.tensor_copy(
                        vst[:, :, :, 64],
                        ones16[:].rearrange("p (a b) -> p a b", a=8))
                    nc.vector.memset(vst[:, :, :, 65], 0.0)
                    nc.sync.dma_start(
                        vaug_d[:, tt, :, :, :]
                        .rearrange("ct p b c -> p ct b c"), vst[:])

            # ============ P2b: all 16 attention blocks ============
            with ExitStack() as p2b:
                vg_p = p2b.enter_context(tc.tile_pool(name="vg", bufs=2))
                pt_p = p2b.enter_context(tc.tile_pool(name="pt2", bufs=3))
                ps_s = p2b.enter_context(tc.tile_pool(name="ps_s2", bufs=1,
                                                      space="PSUM"))
                ps_o = p2b.enter_context(tc.tile_pool(name="ps_o2", bufs=1,
                                                      space="PSUM"))

                def vload(pair):
                    vg = vg_p.tile([128, TT, 2, 66], BF16, tag="vg")
                    nc.sync.dma_start(
                        vg[:], vaug_d[pair, :, :, :, :]
                        .rearrange("tt p b c -> p tt b c"))
                    return vg

                vg = vload(0)
                norm_pend = []
                for pair in range(CT):
                    vg_next = vload(pair + 1) if pair + 1 < CT else None
                    po0 = ps_o.tile([66, NQ], dt.float32, tag="po0")
                    po1 = ps_o.tile([66, NQ], dt.float32, tag="po1")
                    prev = None
                    for kt in range(TT):
                        c0, c1 = s_pair(ps_s, pt_p, pair, kt)
                        if prev is not None:
                            pv(po0, vg[:, kt - 1, 0, :], kt - 1, prev[0])
                            pv(po1, vg[:, kt - 1, 1, :], kt - 1, prev[1])
                        prev = (c0, c1)
                    pv(po0, vg[:, TT - 1, 0, :], TT - 1, prev[0])
                    pv(po1, vg[:, TT - 1, 1, :], TT - 1, prev[1])
                    finish_block(po0, pair, 0)
                    finish_block(po1, pair, 1)
                    norm_pend.append((pair, 0))
                    norm_pend.append((pair, 1))
                    # batch-A reciprocals once sums idx 0-7 (pairs 0-3) ready
                    if pair == 3:
                        # exact DVE reciprocal: off the ACT exp stream and
                        # no ln/exp table switches mid-attention
                        with nc.allow_low_precision(
                                reason="1/sums feeds bf16 rank-1 bcast"):
                            nc.vector.reciprocal(recipsr[:, :],
                                                 st48[0:8, :])
                    elif pair > 3:
                        # drain 2-3 batch-A normalize blocks per pair gap
                        for _ in range(3 if pair > 5 else 2):
                            if norm_pend and norm_pend[0][0] <= 3:
                                p_, h_ = norm_pend.pop(0)
                                norm_block(ps_o, p_, h_, recipsr)
                    vg = vg_next

            # ==== P3: batch-B normalize overlapped with projection ====
            with ExitStack() as p3:
                os_p = p3.enter_context(tc.tile_pool(name="os", bufs=2))
                ps_b = p3.enter_context(tc.tile_pool(name="psb", bufs=1,
                                                     space="PSUM"))
                ps_p3 = p3.enter_context(tc.tile_pool(name="ps3", bufs=1,
                                                      space="PSUM"))

                def proj_mms(ps, tq, oc, ct_lo, ct_hi):
                    for ct in range(ct_lo, ct_hi):
                        nc.tensor.matmul(
                            ps[:], oTf[:, ct, ts(tq, 128)],
                            wp[:, ct, ts(oc, 512)],
                            start=(ct == 0),
                            stop=(not with_bias and ct == CT - 1),
                            skip_group_check=True)
                    if with_bias and ct_hi == CT:
                        nc.tensor.matmul(
                            ps[:], ones1b[:], bproj_t[:, ts(oc, 512)],
                            start=False, stop=True, skip_group_check=True)

                def proj_finish(ps, ost, tq, oc):
                    nc.vector.tensor_copy(ost[:, ts(oc, 512)], ps[:])
                    if oc == 1:
                        nc.sync.dma_start(out[ts(tq, 128), :], ost[:])

                # tq 0-1: pairs-0-3 contraction first, emitted ahead of the
                # batch-B reciprocal/normalize so the PE isn't gated on it
                held = {}
                for tq in range(2):
                    ost = os_p.tile([128, C], dt.float32, tag=f"ost{tq}")
                    for oc in range(2):
                        ps = ps_p3.tile([128, 512], dt.float32,
                                        tag=f"c{tq}{oc}")
                        proj_mms(ps, tq, oc, 0, 4)
                        held[(tq, oc)] = (ps, ost)

                # batch-B reciprocals (pairs 4-7, sums in stB)
                with nc.allow_low_precision(
                        reason="1/sums feeds bf16 rank-1 bcast"):
                    nc.vector.reciprocal(recipsB[:, :], stB[:, :])
                for p_, h_ in norm_pend:
                    norm_block(ps_b, p_, h_,
                               recipsr if p_ * 2 + h_ < 8 else recipsB)

                for tq in range(2):
                    for oc in range(2):
                        ps, ost = held[(tq, oc)]
                        proj_mms(ps, tq, oc, 4, CT)
                        proj_finish(ps, ost, tq, oc)
                for tq in range(2, TQ):
                    ost = os_p.tile([128, C], dt.float32, tag=f"ost{tq % 2}")
                    for oc in range(2):
                        ps = ps_p3.tile([128, 512], dt.float32,
                                        tag=f"c{tq % 2}{oc}")
                        proj_mms(ps, tq, oc, 0, CT)
                        proj_finish(ps, ost, tq, oc)

    nc.compile()
    return nc


_NC = None
_NC_BIAS = None


def _get_nc():
    global _NC
    if _NC is None:
        _NC = build_nc(with_bias=False, trivial_affine=True)
    return _NC


def _get_nc_bias():
    global _NC_BIAS
    if _NC_BIAS is None:
        _NC_BIAS = build_nc(with_bias=True)
    return _NC_BIAS


def _shard_inputs(inputs):
    x = np.asarray(inputs["x"], dtype=np.float32).astype(ml_dtypes.bfloat16)
    shared = {
        "wqkv": np.asarray(inputs["W_qkv"]).astype(ml_dtypes.bfloat16),
        "wprojb": np.asarray(inputs["W_proj"]).astype(ml_dtypes.bfloat16),
        "bqkv": np.asarray(inputs["b_qkv"], dtype=np.float32),
        "bprojb": np.asarray(inputs["b_proj"]).astype(ml_dtypes.bfloat16),
        "ones128": np.ones(128, dtype=np.float32),
        "kron48": np.kron(np.eye(16, dtype=np.float32),
                          np.ones((1, 64), dtype=np.float32))
        .astype(ml_dtypes.bfloat16),
        "gq": np.asarray(inputs["q_gamma"], dtype=np.float32) * np.float32(SCALE),
        "bq": np.asarray(inputs["q_beta"], dtype=np.float32) * np.float32(SCALE),
        "gk": np.asarray(inputs["k_gamma"], dtype=np.float32),
        "bk": np.asarray(inputs["k_beta"], dtype=np.float32),
    }
    in_maps = []
    for core in range(8):
        b, half = core // 2, core % 2
        # xTt[tt, p, kt*128+j] = x[b].T[kt*128+p, tt*128+j]
        xt4 = x[b].T.reshape(CT, 128, TT, 128)
        xtt = np.ascontiguousarray(xt4.transpose(2, 1, 0, 3).reshape(TT, 128, C))
        m = dict(shared)
        m["xTt"] = xtt
        m["xTqt"] = np.ascontiguousarray(
            xtt[half * TQ:(half + 1) * TQ])
        in_maps.append(m)
    return in_maps


def kernel(**inputs) -> np.ndarray:
    from concourse.bass_utils import run_bass_kernel_spmd
    zero_bias = (not np.any(np.asarray(inputs["b_qkv"]))
                 and not np.any(np.asarray(inputs["b_proj"])))
    trivial = (np.all(np.asarray(inputs["q_gamma"]) == 1)
               and np.all(np.asarray(inputs["k_gamma"]) == 1)
               and not np.any(np.asarray(inputs["q_beta"]))
               and not np.any(np.asarray(inputs["k_beta"])))
    nc = (_get_nc() if zero_bias and trivial
          else build_nc(with_bias=not zero_bias, trivial_affine=trivial))
    in_maps = _shard_inputs(inputs)
    res = run_bass_kernel_spmd(nc, in_maps, core_ids=list(range(8)))
    out = np.empty((B, N, C), dtype=np.float32)
    for core in range(8):
        b, half = core // 2, core % 2
        out[b, half * NQ:(half + 1) * NQ, :] = res.results[core]["out"]
    return out



# revision 37
# speedup vs baseline: 1.1051x; 1.0138x over previous
"""MultiHeadSelfAttention (qk-LayerNorm variant) on 8 TRN2 NeuronCores. v4.

v4 changes (vs v2, 728us -> ~540us):
  * P1 LN: rstd = reciprocal(sqrt(var+eps)) -- ACT Sqrt (the ONLY ACT
    func in P1: one table set, no thrash; walrus binds each activation to
    the FIRST set containing it, so an ln/exp pair thrashes 2.6us/tile)
    + exact DVE reciprocal (bass-sanctioned rsqrt). The attention scale
    folds into Sqrt's scale/bias: sqrt(var/S^2 + eps/S^2).
  * P1 pipelining: bn_stats emitted right after each tile's GEMM; the
    rstd/normalize/transpose tail deferred 2 tiles (psum bufs=3) so the
    PE transposes never wait on the DVE chain.
  * Softmax reciprocals on DVE (exact iterative divide) instead of ACT
    ln/exp: total ACT table loads 52 -> 2.
  * kp/qnT bf16; x and W_qkv host-cast to bf16 (GEMM in/out DMA halved;
    rel err 4.8e-3 -> 6.2e-3, budget 2e-2).
  * Weight prefetch: wq/wk/wv share one rep-scope pool (bufs=2); wk loads
    behind Q-phase traffic, wv as a 3rd tag generation whose WAR makes it
    land during the K phase; wp (proj) prefetched from P2a on.
  * P2a is V-GEMM only (the old fused pair0 attention stretched the
    PE-bound V phase without filling ACT); all 8 pairs run in P2b at the
    exp cadence floor (~1.09us/exp measured, <4% stall).
  * Softmax normalize overlapped: batch-A recips after pair 3, pairs 0-3
    normalized inside later pair gaps via po-tag reuse; batch B + pairs
    4-7 normalize in P3, overlapped with the ct<4 half of the first
    projection tiles (split-contraction psum accumulation).

Problem (B=4, N=2048, C=1024, H=16, D=64, fp32):
    qkv = x @ W_qkv + b_qkv ; q,k,v = split(qkv)
    q = LN(q)*scale ; k = LN(k)          (LN over full C)
    attn = softmax(q_h @ k_h^T) per head ; o = attn @ v_h
    out = concat_heads(o) @ W_proj + b_proj

Sharding: core i handles batch i//2 and query-half i%2 (1024 query rows).
Each core computes K/V for the full sequence of its batch. No collectives.

v2 design notes (vs v1):
  * ACT (scalar engine) runs ONLY Exp/Log: softmax exp is the hard floor
    (1 elem/cycle/lane, ~285us/core), so LN stats moved to DVE bn_stats
    and rstd/reciprocal computed as exp(-0.5*ln(var+eps)) / exp(-ln(s))
    on ACT -- all in the natural_log_exp table set, zero table thrash.
  * K^T SBUF-resident (64KB/part); V staged to DRAM as bf16 aug tiles
    [64 v | ones | 0] (66 wide, even for bf16 ISA), reloaded per pair.
  * S^T matmuls fp32r with h0 (rows 0-63) / h1 (rows 64-127) issued
    back-to-back: PE row-group concurrency gives ~2x (measured 147ns/MM).
  * PV + projection in bf16 (safe: post-softmax averaging; numerator and
    denominator share pT rounding).
  * pair0-h0 attention fused into the V-production phase so exp starts
    while V tiles are still being produced.
  * Softmax normalization deferred: unnormalized O^T and row-sums are
    saved per (pair,h2); reciprocals batched on ACT, broadcast via K=1
    rank-1 matmuls, applied with one DVE mult per head before proj.
"""
import numpy as np
import ml_dtypes
from contextlib import ExitStack

import concourse.bass as bass


# NOTE: do NOT override BASS_ACT_ROOT_JSON_PATH to merge the ln/exp table
# sets -- the runtime resolves PSEUDO_LOAD_ACT_FUNC_SET against its own
# stock registry and a reordered json hangs the NEFF (mesh desync).
from concourse import bacc
import concourse.tile as tile
import concourse.mybir as mybir
from concourse.masks import make_identity

dt = mybir.dt
AF = mybir.ActivationFunctionType
OP = mybir.AluOpType
ts = bass.ts

B, N, C = 4, 2048, 1024
H, D = 16, 64
NQ = 1024            # query rows per core
SCALE = D ** -0.5
EPS = 1e-6
TT = N // 128        # 16 token tiles (full seq)
TQ = NQ // 128       # 8 token tiles (query half)
CT = C // 128        # 8 channel tiles (= head pairs)
F32R = dt.float32r
BF16 = dt.bfloat16


def build_nc(reps=1, with_bias=False, trivial_affine=True):
    nc = bacc.Bacc()
    xTt = nc.dram_tensor("xTt", [TT, 128, C], BF16, kind="ExternalInput")
    xTqt = nc.dram_tensor("xTqt", [TQ, 128, C], BF16, kind="ExternalInput")
    wqkv = nc.dram_tensor("wqkv", [C, 3 * C], BF16, kind="ExternalInput")
    wprojb = nc.dram_tensor("wprojb", [C, C], BF16, kind="ExternalInput")
    bqkv = nc.dram_tensor("bqkv", [3 * C], dt.float32, kind="ExternalInput")
    bprojb = nc.dram_tensor("bprojb", [C], BF16, kind="ExternalInput")
    ones128 = nc.dram_tensor("ones128", [128], dt.float32, kind="ExternalInput")
    kron48 = nc.dram_tensor("kron48", [16, 1024], BF16, kind="ExternalInput")
    gq = nc.dram_tensor("gq", [C], dt.float32, kind="ExternalInput")
    bq = nc.dram_tensor("bq", [C], dt.float32, kind="ExternalInput")
    gk = nc.dram_tensor("gk", [C], dt.float32, kind="ExternalInput")
    bk = nc.dram_tensor("bk", [C], dt.float32, kind="ExternalInput")
    out = nc.dram_tensor("out", [NQ, C], dt.float32, kind="ExternalOutput")

    with tile.TileContext(nc) as tc, ExitStack() as top:
        const = top.enter_context(tc.tile_pool(name="const", bufs=1))
        dram = top.enter_context(tc.tile_pool(name="dram", bufs=1, space="DRAM"))
        res = top.enter_context(tc.tile_pool(name="res", bufs=1))

        # ---- constants ----
        ident = const.tile([128, 128], dt.float32)
        make_identity(nc, ident[:])
        identb = const.tile([128, 128], BF16)
        nc.vector.tensor_copy(identb[:], ident[:])
        ones1 = const.tile([1, 128], F32R)
        nc.sync.dma_start(ones1[:], ones128.rearrange("(o n) -> o n", o=1)
                          .bitcast(F32R))
        ones1b = const.tile([1, 128], BF16)
        nc.vector.tensor_copy(ones1b[:], ones1[:].bitcast(dt.float32))
        ones16 = const.tile([128, 16], dt.float32)
        nc.vector.memset(ones16[:], 1.0)
        eps_t = const.tile([128, 1], dt.float32)
        nc.vector.memset(eps_t[:], EPS)
        epsq_t = const.tile([128, 1], dt.float32)
        nc.vector.memset(epsq_t[:], EPS / SCALE ** 2)
        gq_t = const.tile([128, CT], dt.float32)
        bq_t = const.tile([128, CT], dt.float32)
        gk_t = const.tile([128, CT], dt.float32)
        bk_t = const.tile([128, CT], dt.float32)
        for t_, d_ in ((gq_t, gq), (bq_t, bq), (gk_t, gk), (bk_t, bk)):
            nc.sync.dma_start(t_[:], d_.rearrange("(ct p) -> p ct", p=128))
        if with_bias:
            bqkv_t = const.tile([1, 3 * C], F32R)
            nc.sync.dma_start(bqkv_t[:],
                              bqkv.rearrange("(o n) -> o n", o=1).bitcast(F32R))
            bproj_t = const.tile([1, C], BF16)
            nc.sync.dma_start(bproj_t[:],
                              bprojb.rearrange("(o n) -> o n", o=1))
        kron_t = const.tile([16, 1024], BF16)
        nc.sync.dma_start(kron_t[:], kron48[:, :])

        # ---- resident tensors ----
        kp = res.tile([128, CT, N], BF16)           # K^T, LN'd [c, t]
        qnT = res.tile([128, CT, NQ], BF16)         # Q^T, LN'd+scaled [c, t]
        oTr = res.tile([128, CT, NQ], BF16)         # O^T unnormalized
        oTf = res.tile([128, CT, NQ], BF16)         # O^T normalized
        # rows 0-7: softmax row-sums idx 0-7; 32-47: ln scratch
        # (engine operand partition bases must be 32-aligned)
        st48 = res.tile([48, NQ], dt.float32)
        stB = res.tile([8, NQ], dt.float32)         # sums idx 8-15 (aligned base)
        recipsr = res.tile([8, NQ], BF16)           # 1/sums idx 0-7
        recipsB = res.tile([8, NQ], BF16)           # 1/sums idx 8-15
        stmp = res.tile([1, NQ], dt.float32)        # sums-row staging

        # ---- DRAM staging: V-aug per pair ----
        vaug_d = dram.tile([CT, TT, 128, 2, 66], BF16)

        def load_w_group(wq_p, oc_base):
            """[128, CT, C] f32r tile with W_qkv[:, oc_base:oc_base+C].
            One DMA per ic-tile: chunks land on parallel queues and the
            first GEMM starts as soon as chunk 0 arrives."""
            w_t = wq_p.tile([128, CT, C], BF16, tag="w_t")
            wr = wqkv.rearrange("(kt p) c -> p kt c", p=128)
            for kt in range(CT):
                nc.sync.dma_start(w_t[:, kt, :],
                                  wr[:, kt, oc_base:oc_base + C])
            return w_t

        def qkv_psum(ps, x_tile, w_t, oc_base):
            """accumulate x_tile.T @ Wgroup (+ bias) per 512-chunk."""
            for ch in range(2):
                for kt in range(CT):
                    nc.tensor.matmul(
                        ps[:, ts(ch, 512)],
                        x_tile[:, kt, :],
                        w_t[:, kt, ts(ch, 512)],
                        start=(kt == 0),
                        stop=(not with_bias and kt == CT - 1),
                        skip_group_check=True)
                if with_bias:
                    lo = oc_base + ch * 512
                    nc.tensor.matmul(
                        ps[:, ts(ch, 512)], ones1[:],
                        bqkv_t[:, lo:lo + 512],
                        start=False, stop=True, skip_group_check=True)

        def ln_stats(ln_p, ps_tok):
            """DVE bn stats -> mv [128, 2] (mean, var) per token."""
            st6 = ln_p.tile([128, 2, 6], dt.float32, tag="st6")
            nc.vector.bn_stats(st6[:, 0, :], ps_tok[:, 0:512])
            nc.vector.bn_stats(st6[:, 1, :], ps_tok[:, 512:1024])
            mv = ln_p.tile([128, 2], dt.float32, tag="mv")
            nc.vector.bn_aggr(mv[:], st6[:])
            return mv

        def ln_rstd(ln_p, mv, exp_bias):
            """rstd = exp(-0.5*ln(var+eps) + exp_bias) on ACT (ln/exp set;
            exp_bias = ln(scale) tile folds the attention scale for Q).
            General-affine path only: the ln->exp pair thrashes ACT table
            sets (~2.6us/tile), so the trivial path uses ln_rstd_gps."""
            lnv = ln_p.tile([128, 1], dt.float32, tag="lnv")
            nc.scalar.activation(lnv[:], mv[:, 1:2], AF.Ln, bias=eps_t[:])
            rstd = ln_p.tile([128, 1], dt.float32, tag="rstd")
            nc.scalar.activation(rstd[:], lnv[:], AF.Exp, scale=-0.5,
                                 bias=exp_bias)
            return rstd

        def ln_rstd_triv(ln_p, mv, sbias):
            """rstd = scale/sqrt(var+eps) via ACT Sqrt (the only ACT func
            used in P1 -- single table set, no thrash) + exact DVE
            reciprocal (the bass-sanctioned rsqrt pattern). The attention
            scale folds in as sqrt(var/S^2 + eps/S^2) = sqrt(var+eps)/S."""
            sq = ln_p.tile([128, 1], dt.float32, tag="sq")
            nc.scalar.activation(sq[:], mv[:, 1:2], AF.Sqrt,
                                 scale=sbias[0], bias=sbias[1])
            rstd = ln_p.tile([128, 1], dt.float32, tag="rstd")
            nc.vector.reciprocal(rstd[:], sq[:])
            return rstd

        def ln_tail_triv(ln_p, tok_p, pst_p, ps_tok, mv, sbias, sinkT,
                         tslot):
            """deferred LN tail: rstd via ACT Sqrt + DVE reciprocal (single
            ACT table set in P1), normalize on DVE writing bf16 tok,
            grouped bf16 PE transposes, DVE sinks. bn_stats for the tile
            ran right after its GEMM so this tail's tensor_scalar is never
            queued behind a full DVE chain."""
            rstd = ln_rstd_triv(ln_p, mv, sbias)
            tok = tok_p.tile([128, C], BF16, tag="tok")
            nc.vector.tensor_scalar(tok[:], ps_tok[:], mv[:, 0:1], rstd[:],
                                    op0=OP.subtract, op1=OP.mult)
            for g in range(2):
                ps_t = pst_p.tile([128, 512], BF16, tag="ps_t")
                for i in range(4):
                    nc.tensor.matmul(ps_t[:, ts(i, 128)],
                                     tok[:, ts(g * 4 + i, 128)], identb[:],
                                     is_transpose=True, start=True, stop=True,
                                     skip_group_check=True)
                nc.vector.tensor_copy(
                    sinkT[:, g * 4:g * 4 + 4, ts(tslot, 128)],
                    ps_t[:].rearrange("p (i t) -> p i t", i=4))

        def ln_transpose(ln_p, pst_p, ps_tok, exp_bias, g_t, b_t, sink):
            """general affine: per-ct PE transpose + gamma/beta fold in sink."""
            mv = ln_stats(ln_p, ps_tok)
            rstd = ln_rstd(ln_p, mv, exp_bias)
            tok = ln_p.tile([128, C], dt.float32, tag="tok")
            nc.vector.tensor_scalar(tok[:], ps_tok[:], mv[:, 0:1], rstd[:],
                                    op0=OP.subtract, op1=OP.mult)
            for ct in range(CT):
                ps_t = pst_p.tile([128, 128], dt.float32, tag="ps_t")
                nc.tensor.matmul(ps_t[:], tok[:, ts(ct, 128)], ident[:],
                                 is_transpose=True, start=True, stop=True,
                                 skip_group_check=True)
                sink(ct, ps_t, g_t[:, ct:ct + 1], b_t[:, ct:ct + 1])

        def s_exp(ps_s, pt_p, pair, kt, tp, ptag=None):
            """S^T for one head (64 contraction rows at tp) + wide exp."""
            h = tp // 64
            pss = ps_s.tile([128, NQ], dt.float32, tag=ptag or f"pss{h}")
            for qc in range(2):
                nc.tensor.matmul(
                    pss[:, ts(qc, 512)],
                    kp[tp:tp + 64, pair, ts(kt, 128)],
                    qnT[tp:tp + 64, pair, ts(qc, 512)],
                    start=True, stop=True, skip_group_check=True)
            pT = pt_p.tile([128, NQ], BF16, tag=f"pt{h}")
            nc.scalar.activation(pT[:], pss[:], AF.Exp)
            return pT

        def s_pair(ps_s, pt_p, pair, kt):
            """both heads' S^T, h0/h1 interleaved for PE row-group
            concurrency, then one wide exp per head."""
            pss0 = ps_s.tile([128, NQ], dt.float32, tag="pss0")
            pss1 = ps_s.tile([128, NQ], dt.float32, tag="pss1")
            for qc in range(2):
                nc.tensor.matmul(
                    pss0[:, ts(qc, 512)], kp[0:64, pair, ts(kt, 128)],
                    qnT[0:64, pair, ts(qc, 512)],
                    start=True, stop=True, skip_group_check=True)
                nc.tensor.matmul(
                    pss1[:, ts(qc, 512)], kp[64:128, pair, ts(kt, 128)],
                    qnT[64:128, pair, ts(qc, 512)],
                    start=True, stop=True, skip_group_check=True)
            pT0 = pt_p.tile([128, NQ], BF16, tag="pt0")
            nc.scalar.activation(pT0[:], pss0[:], AF.Exp)
            pT1 = pt_p.tile([128, NQ], BF16, tag="pt1")
            nc.scalar.activation(pT1[:], pss1[:], AF.Exp)
            return pT0, pT1

        def pv(po, vsrc, kt, pT):
            for qc in range(2):
                nc.tensor.matmul(
                    po[:, ts(qc, 512)], vsrc,
                    pT[:, ts(qc, 512)],
                    start=(kt == 0), stop=(kt == TT - 1),
                    skip_group_check=True)

        def finish_block(po, pair, h2):
            """save row-sums + unnormalized O^T for (pair, h2)."""
            idx = pair * 2 + h2
            nc.vector.tensor_copy(stmp[:], po[64:65, :])
            if idx < 8:
                nc.sync.dma_start(st48[idx:idx + 1, :], stmp[:])
            else:
                nc.sync.dma_start(stB[idx - 8:idx - 7, :], stmp[:])
            nc.vector.tensor_copy(oTr[h2 * 64:h2 * 64 + 64, pair, :],
                                  po[0:64, :])

        def norm_block(ps_o, pair, h2, rtile):
            """oTf = oTr * bcast(1/sums) for one (pair, h2) block.
            bc reuses the po psum tags (pool is full during attention);
            the resulting WAR chain orders it after the block's finish."""
            idx = pair * 2 + h2
            ridx = idx % 8
            bc = ps_o.tile([66, NQ], dt.float32, tag=f"po{h2}")
            for qc in range(2):
                nc.tensor.matmul(
                    bc[0:64, ts(qc, 512)],
                    kron_t[0:8, ridx * 64:ridx * 64 + 64],
                    rtile[0:8, ts(qc, 512)],
                    start=True, stop=True, skip_group_check=True)
            nc.vector.tensor_tensor(
                oTf[h2 * 64:h2 * 64 + 64, pair, :],
                oTr[h2 * 64:h2 * 64 + 64, pair, :], bc[0:64, :],
                op=OP.mult)

        for _rep in range(reps):
            # ============ P1a: Q group (query half) ============
            if _rep == 0:
                wq_p = top.enter_context(tc.tile_pool(name="wq", bufs=2))
            with ExitStack() as p1:
                xt_p = p1.enter_context(tc.tile_pool(name="xt", bufs=2))
                ln_p = p1.enter_context(tc.tile_pool(name="ln", bufs=3))
                tok_p = p1.enter_context(tc.tile_pool(name="tok", bufs=3))
                ps_p = p1.enter_context(tc.tile_pool(name="ps1", bufs=3,
                                                     space="PSUM"))
                pst_p = p1.enter_context(tc.tile_pool(name="pst", bufs=2,
                                                      space="PSUM"))

                # LN emits are deferred 2 GEMMs back so the stats/rstd/
                # normalize chain (~4.5us latency) never stalls the PE
                # transposes; psum bufs=3 covers the 3 live generations.
                pend = []

                def drain_pend(limit):
                    while len(pend) > limit:
                        pend.pop(0)()

                # first x tile ahead of the weight chunks so the queue
                # doesn't make the first GEMM wait behind all of W
                xt0 = xt_p.tile([128, CT, 128], BF16, tag="xt")
                nc.sync.dma_start(xt0[:].rearrange("p a b -> p (a b)"),
                                  xTqt[0])
                wq = load_w_group(wq_p, 0)
                wk = None
                for tq in range(TQ):
                    if tq == 0:
                        xt = xt0
                    else:
                        xt = xt_p.tile([128, CT, 128], BF16, tag="xt")
                        nc.sync.dma_start(
                            xt[:].rearrange("p a b -> p (a b)"),
                            xTqt[tq])
                    ps_q = ps_p.tile([128, C], dt.float32, tag="ps")
                    qkv_psum(ps_q, xt, wq, 0)
                    if tq == 0:
                        # prefetch K weights behind the Q-phase traffic
                        wk = load_w_group(wq_p, C)

                    def q_sink(ct, ps_t, g, b, tq=tq):
                        nc.vector.tensor_scalar(
                            qnT[:, ct, ts(tq, 128)], ps_t[:], g, b,
                            op0=OP.mult, op1=OP.add)

                    if trivial_affine:
                        mv_q = ln_stats(ln_p, ps_q)

                        def q_emit(ps_q=ps_q, tq=tq, mv_q=mv_q):
                            ln_tail_triv(ln_p, tok_p, pst_p, ps_q, mv_q,
                                         (1.0 / SCALE ** 2, epsq_t[:]),
                                         qnT, tq)
                    else:
                        def q_emit(ps_q=ps_q, tq=tq, q_sink=q_sink):
                            ln_transpose(ln_p, pst_p, ps_q, 0.0,
                                         gq_t, bq_t, q_sink)
                    pend.append(q_emit)
                    drain_pend(2)

                # prefetch V weights as the 3rd w_t generation: the WAR on
                # wq's buffer makes the DMA start as Q-phase ends, landing
                # well inside the K phase
                wv = load_w_group(wq_p, 2 * C)

                # ============ P1b: K group (full seq) ============
                for tt in range(TT):
                    xt = xt_p.tile([128, CT, 128], BF16, tag="xt")
                    nc.sync.dma_start(
                        xt[:].rearrange("p a b -> p (a b)"),
                        xTt[tt])
                    ps_k = ps_p.tile([128, C], dt.float32, tag="ps")
                    qkv_psum(ps_k, xt, wk, C)

                    def k_sink(ct, ps_t, g, b, tt=tt):
                        nc.vector.tensor_scalar(
                            kp[:, ct, ts(tt, 128)], ps_t[:], g, b,
                            op0=OP.mult, op1=OP.add)

                    if trivial_affine:
                        mv_k = ln_stats(ln_p, ps_k)

                        def k_emit(ps_k=ps_k, tt=tt, mv_k=mv_k):
                            ln_tail_triv(ln_p, tok_p, pst_p, ps_k, mv_k,
                                         (1.0, eps_t[:]), kp, tt)
                    else:
                        def k_emit(ps_k=ps_k, tt=tt, k_sink=k_sink):
                            ln_transpose(ln_p, pst_p, ps_k, 0.0,
                                         gk_t, bk_t, k_sink)
                    pend.append(k_emit)
                    drain_pend(2)
                drain_pend(0)

            # prefetch the projection weights early (2MB; used in P3)
            if _rep == 0:
                wp_p = top.enter_context(tc.tile_pool(name="wp", bufs=1))
            wp = wp_p.tile([128, CT, C], BF16, tag="wp")
            nc.sync.dma_start(wp[:],
                              wprojb.rearrange("(kt p) c -> p kt c", p=128))

            # ============ P2a: V group (GEMM-bound, no fusion) ============
            with ExitStack() as p2a:
                xt_p = p2a.enter_context(tc.tile_pool(name="xt2", bufs=2))
                st_p = p2a.enter_context(tc.tile_pool(name="st", bufs=2))
                ps_p = p2a.enter_context(tc.tile_pool(name="psv", bufs=2,
                                                      space="PSUM"))

                for tt in range(TT):
                    xt = xt_p.tile([128, CT, 128], BF16, tag="xt")
                    nc.sync.dma_start(
                        xt[:].rearrange("p a b -> p (a b)"),
                        xTt[tt])
                    ps_v = ps_p.tile([128, C], dt.float32, tag="ps")
                    qkv_psum(ps_v, xt, wv, 2 * C)
                    vst = st_p.tile([128, CT, 2, 66], BF16, tag="vst")
                    for half8 in range(2):
                        nc.vector.tensor_copy(
                            vst[:, half8 * 4:half8 * 4 + 4, :, 0:64],
                            ps_v[:, ts(half8, 512)]
                            .rearrange("p (pr b c) -> p pr b c", pr=4, b=2))
                    nc.vector.tensor_copy(
                        vst[:, :, :, 64],
                        ones16[:].rearrange("p (a b) -> p a b", a=8))
                    nc.vector.memset(vst[:, :, :, 65], 0.0)
                    nc.sync.dma_start(
                        vaug_d[:, tt, :, :, :]
                        .rearrange("ct p b c -> p ct b c"), vst[:])

            # ============ P2b: all 16 attention blocks ============
            with ExitStack() as p2b:
                vg_p = p2b.enter_context(tc.tile_pool(name="vg", bufs=2))
                pt_p = p2b.enter_context(tc.tile_pool(name="pt2", bufs=3))
                ps_s = p2b.enter_context(tc.tile_pool(name="ps_s2", bufs=1,
                                                      space="PSUM"))
                ps_o = p2b.enter_context(tc.tile_pool(name="ps_o2", bufs=1,
                                                      space="PSUM"))

                def vload(pair):
                    vg = vg_p.tile([128, TT, 2, 66], BF16, tag="vg")
                    nc.sync.dma_start(
                        vg[:], vaug_d[pair, :, :, :, :]
                        .rearrange("tt p b c -> p tt b c"))
                    return vg

                vg = vload(0)
                norm_pend = []
                for pair in range(CT):
                    vg_next = vload(pair + 1) if pair + 1 < CT else None
                    po0 = ps_o.tile([66, NQ], dt.float32, tag="po0")
                    po1 = ps_o.tile([66, NQ], dt.float32, tag="po1")
                    prev = None
                    for kt in range(TT):
                        c0, c1 = s_pair(ps_s, pt_p, pair, kt)
                        if prev is not None:
                            pv(po0, vg[:, kt - 1, 0, :], kt - 1, prev[0])
                            pv(po1, vg[:, kt - 1, 1, :], kt - 1, prev[1])
                        prev = (c0, c1)
                    pv(po0, vg[:, TT - 1, 0, :], TT - 1, prev[0])
                    pv(po1, vg[:, TT - 1, 1, :], TT - 1, prev[1])
                    finish_block(po0, pair, 0)
                    finish_block(po1, pair, 1)
                    norm_pend.append((pair, 0))
                    norm_pend.append((pair, 1))
                    # batch-A reciprocals once sums idx 0-7 (pairs 0-3) ready
                    if pair == 3:
                        # exact DVE reciprocal: off the ACT exp stream and
                        # no ln/exp table switches mid-attention
                        with nc.allow_low_precision(
                                reason="1/sums feeds bf16 rank-1 bcast"):
                            nc.vector.reciprocal(recipsr[:, :],
                                                 st48[0:8, :])
                    elif pair > 3:
                        # drain 2-3 batch-A normalize blocks per pair gap
                        for _ in range(3 if pair > 5 else 2):
                            if norm_pend and norm_pend[0][0] <= 3:
                                p_, h_ = norm_pend.pop(0)
                                norm_block(ps_o, p_, h_, recipsr)
                    vg = vg_next

            # ==== P3: batch-B normalize overlapped with projection ====
            with ExitStack() as p3:
                os_p = p3.enter_context(tc.tile_pool(name="os", bufs=2))
                ps_b = p3.enter_context(tc.tile_pool(name="psb", bufs=1,
                                                     space="PSUM"))
                ps_p3 = p3.enter_context(tc.tile_pool(name="ps3", bufs=1,
                                                      space="PSUM"))

                def proj_mms(ps, tq, oc, ct_lo, ct_hi):
                    for ct in range(ct_lo, ct_hi):
                        nc.tensor.matmul(
                            ps[:], oTf[:, ct, ts(tq, 128)],
                            wp[:, ct, ts(oc, 512)],
                            start=(ct == 0),
                            stop=(not with_bias and ct == CT - 1),
                            skip_group_check=True)
                    if with_bias and ct_hi == CT:
                        nc.tensor.matmul(
                            ps[:], ones1b[:], bproj_t[:, ts(oc, 512)],
                            start=False, stop=True, skip_group_check=True)

                def proj_finish(ps, ost, tq, oc):
                    nc.vector.tensor_copy(ost[:, ts(oc, 512)], ps[:])
                    if oc == 1:
                        nc.sync.dma_start(out[ts(tq, 128), :], ost[:])

                # tq 0-1: pairs-0-3 contraction first, emitted ahead of the
                # batch-B reciprocal/normalize so the PE isn't gated on it
                held = {}
                for tq in range(2):
                    ost = os_p.tile([128, C], dt.float32, tag=f"ost{tq}")
                    for oc in range(2):
                        ps = ps_p3.tile([128, 512], dt.float32,
                                        tag=f"c{tq}{oc}")
                        proj_mms(ps, tq, oc, 0, 4)
                        held[(tq, oc)] = (ps, ost)

                # batch-B reciprocals (pairs 4-7, sums in stB)
                with nc.allow_low_precision(
                        reason="1/sums feeds bf16 rank-1 bcast"):
                    nc.vector.reciprocal(recipsB[:, :], stB[:, :])
                for p_, h_ in norm_pend:
                    norm_block(ps_b, p_, h_,
                               recipsr if p_ * 2 + h_ < 8 else recipsB)

                for tq in range(2):
                    for oc in range(2):
                        ps, ost = held[(tq, oc)]
                        proj_mms(ps, tq, oc, 4, CT)
                        proj_finish(ps, ost, tq, oc)
                for tq in range(2, TQ):
                    ost = os_p.tile([128, C], dt.float32, tag=f"ost{tq % 2}")
                    for oc in range(2):
                        ps = ps_p3.tile([128, 512], dt.float32,
                                        tag=f"c{tq % 2}{oc}")
                        proj_mms(ps, tq, oc, 0, CT)
                        proj_finish(ps, ost, tq, oc)

    nc.compile()
    return nc


_NC = None
_NC_BIAS = None


def _get_nc():
    global _NC
    if _NC is None:
        _NC = build_nc(with_bias=False, trivial_affine=True)
    return _NC


def _get_nc_bias():
    global _NC_BIAS
    if _NC_BIAS is None:
        _NC_BIAS = build_nc(with_bias=True)
    return _NC_BIAS


def _shard_inputs(inputs):
    x = np.asarray(inputs["x"], dtype=np.float32).astype(ml_dtypes.bfloat16)
    shared = {
        "wqkv": np.asarray(inputs["W_qkv"]).astype(ml_dtypes.bfloat16),
        "wprojb": np.asarray(inputs["W_proj"]).astype(ml_dtypes.bfloat16),
        "bqkv": np.asarray(inputs["b_qkv"], dtype=np.float32),
        "bprojb": np.asarray(inputs["b_proj"]).astype(ml_dtypes.bfloat16),
        "ones128": np.ones(128, dtype=np.float32),
        "kron48": np.kron(np.eye(16, dtype=np.float32),
                          np.ones((1, 64), dtype=np.float32))
        .astype(ml_dtypes.bfloat16),
        "gq": np.asarray(inputs["q_gamma"], dtype=np.float32) * np.float32(SCALE),
        "bq": np.asarray(inputs["q_beta"], dtype=np.float32) * np.float32(SCALE),
        "gk": np.asarray(inputs["k_gamma"], dtype=np.float32),
        "bk": np.asarray(inputs["k_beta"], dtype=np.float32),
    }
    in_maps = []
    for core in range(8):
        b, half = core // 2, core % 2
        # xTt[tt, p, kt*128+j] = x[b].T[kt*128+p, tt*128+j]
        xt4 = x[b].T.reshape(CT, 128, TT, 128)
        xtt = np.ascontiguousarray(xt4.transpose(2, 1, 0, 3).reshape(TT, 128, C))
        m = dict(shared)
        m["xTt"] = xtt
        m["xTqt"] = np.ascontiguousarray(
            xtt[half * TQ:(half + 1) * TQ])
        in_maps.append(m)
    return in_maps


def kernel(**inputs) -> np.ndarray:
    from concourse.bass_utils import run_bass_kernel_spmd
    zero_bias = (not np.any(np.asarray(inputs["b_qkv"]))
                 and not np.any(np.asarray(inputs["b_proj"])))
    trivial = (np.all(np.asarray(inputs["q_gamma"]) == 1)
               and np.all(np.asarray(inputs["k_gamma"]) == 1)
               and not np.any(np.asarray(inputs["q_beta"]))
               and not np.any(np.asarray(inputs["k_beta"])))
    nc = (_get_nc() if zero_bias and trivial
          else build_nc(with_bias=not zero_bias, trivial_affine=trivial))
    in_maps = _shard_inputs(inputs)
    res = run_bass_kernel_spmd(nc, in_maps, core_ids=list(range(8)))
    out = np.empty((B, N, C), dtype=np.float32)
    for core in range(8):
        b, half = core // 2, core % 2
        out[b, half * NQ:(half + 1) * NQ, :] = res.results[core]["out"]
    return out

